# revision 6
# baseline (speedup 1.0000x reference)
"""CausalAttention2d Trainium2 kernel (8-core SPMD), v4.

Shards (B=2, heads=8) -> 16 (batch, head) pairs across 8 cores: core c handles
batch b=c//4 and head-pair p=c%4 (heads 2p, 2p+1 = E-channels 128p..128p+128).
Each core computes Q/K/V projections for its head pair, causal softmax
attention over the full N=4096 sequence, and writes its [128, 4096] slice of
the channel-major output. Host assembles the full (2, 512, 64, 64) tensor.

v4 changes over v3 (188us):
 - AV matmuls column-tiled: the two heads' AV run CONCURRENTLY on PE column
   halves (tile_position (0,0)/(0,64), M=64 each) into one [128,512] psum
   bank -> 216ns per key-chunk for BOTH heads (was 432, HW-verified).
 - Softmax denominators come from a dedicated den-slot: per group, four
   concurrent M=32 ones-broadcast matmuls (col positions 0/32/64/96) that
   accumulate 32 identical copies of each (head, chunk-parity) partial den
   into a second psum bank; one ACT cross-base evac + one DVE add folds the
   quadrants. Den slot runs BEFORE the AV slots so the last tile's
   denominator roundtrip overlaps its final AV matmuls.
 - Finalize: ACT (not DVE) evacuates AV psum; ONE [128,512] DVE mul
   normalizes both heads; one DMA per tile writes the output.
 - Input staging: biases/mask merged into single contiguous staging tensors
   and weights sent pre-permuted, killing the per-partition 4-byte DMA
   descriptor storm that delayed the first matmul to 14.5us.
 - ~30 tiny warm-up matmuls during the input-DMA wait pre-warm the PE HAM
   clock gate (cold K=4/8 -> warm K=8/8).
 - exp split tuned: head0 exact on ACT; head1 Schraudolph on DVE for cols
   [0:SPL2) of each 512-chunk, exact ACT tail for the rest (engine balance).
"""

import os

import numpy as np
import ml_dtypes

B, C, H, W = 2, 512, 64, 64
N = H * W          # 4096
E = 512
NH = 8
HD = 64
NT = 8             # n-tiles of 512
CCH = 4            # contraction chunks of 128 over C

LN2 = 0.6931471805599453
C1 = 128.0 / LN2           # score prescale (folded into Wq host-side)
C2 = 16248.75              # Schraudolph bias (calibrated, robust to rounding mode)
SPL2 = 480                 # head1 exp: DVE Schraudolph cols [0:SPL2), ACT tail

_cache = {}
last_results = None  # BassKernelResults of the most recent run (for profiling)


def _split_multi_waits(nc, mybir, bass_rust):
    """This walrus build accepts only ONE sync-wait per instruction; hoist
    extra waits onto single-wait NOPs inserted just before, preserving
    per-engine program order."""
    n = 0
    for f in nc.m.functions:
        for bb in f.blocks:
            old = list(bb.instructions)
            new = []
            changed = False
            for inst in old:
                si = inst.sync_info
                if si is not None and si.on_wait and len(si.on_wait) > 1:
                    waits = list(si.on_wait)
                    for w in waits[:-1]:
                        nop = bass_rust.InstNoOp(
                            name=f"I-sw{n}", engine=inst.engine, ins=[], outs=[]
                        )
                        n += 1
                        nop.sync_info = mybir.SyncInfo(on_wait=[w], on_update=[])
                        new.append(nop)
                    inst.sync_info = mybir.SyncInfo(
                        on_wait=[waits[-1]], on_update=list(si.on_update)
                    )
                    changed = True
                new.append(inst)
            if changed:
                bb.instructions = new
    return n


def _build_program():
    import bass_rust
    import concourse.bass as bass
    import concourse.mybir as mybir
    import concourse.tile as tile
    from contextlib import ExitStack

    f32 = mybir.dt.float32
    bf16 = mybir.dt.bfloat16
    i16 = mybir.dt.int16
    Exp = mybir.ActivationFunctionType.Exp
    Ident = mybir.ActivationFunctionType.Identity
    Add = mybir.AluOpType.add

    nc = bass.Bass()
    xq = nc.dram_tensor("xq", [C, N], bf16, kind="ExternalInput")
    xk = nc.dram_tensor("xk", [C, N], bf16, kind="ExternalInput")
    wqkv = nc.dram_tensor("wqkv", [128, 3, CCH, 128], bf16, kind="ExternalInput")
    smalls = nc.dram_tensor("smalls", [128, 130], f32, kind="ExternalInput")
    mask = nc.dram_tensor("mask", [128, 128], bf16, kind="ExternalInput")
    o = nc.dram_tensor("o", [128, N], f32, kind="ExternalOutput")

    with tile.TileContext(nc) as tc, ExitStack() as ctx:
        singles = ctx.enter_context(tc.tile_pool(name="singles", bufs=1))
        ppool = ctx.enter_context(tc.tile_pool(name="ppool", bufs=2))
        opool = ctx.enter_context(tc.tile_pool(name="opool", bufs=3))
        drs = ctx.enter_context(tc.tile_pool(name="drs", bufs=2, space="DRAM"))
        # PSUM budget (8 banks): st ring 3 bufs x 2 banks (scores AND
        # projection psums share it) + av 1 bank + den 1 bank.
        stps = ctx.enter_context(tc.tile_pool(name="stps", bufs=3, space="PSUM"))
        avps = ctx.enter_context(tc.tile_pool(name="avps", bufs=1, space="PSUM"))
        denps = ctx.enter_context(tc.tile_pool(name="denps", bufs=1, space="PSUM"))

        smalls_sb = singles.tile([128, 130], f32)
        mask_sb = singles.tile([128, 128], bf16)
        ones32 = singles.tile([128, 32], bf16)
        warm_sb = singles.tile([128, 16], bf16)
        # warm-up: force the exp ACT table set resident before the first
        # real activation (overlaps the input DMAs instead of serializing
        # into the first tile's dependency chain)
        warm = singles.tile([1, 8], f32)
        nc.vector.memset(warm, 0.0)
        nc.vector.memset(ones32, 1.0)
        nc.vector.memset(warm_sb, 1.0)
        nc.scalar.activation(warm[:, :], warm[:, :], Exp)
        nc.sync.dma_start(out=smalls_sb, in_=smalls[:, :])
        nc.sync.dma_start(out=mask_sb, in_=mask[:, :])
        bq_sb = smalls_sb[:, 0:1]
        bk_sb = smalls_sb[:, 1:2]
        bvb_sb = smalls_sb[:, 2:130]

        # persistent activations
        qt_all = singles.tile([128, N], bf16)   # [2*64 d, q] (prescaled by C1)
        kt_all = singles.tile([128, N], bf16)   # [2*64 d, keys]
        v_all = singles.tile([128, N // 128, 128], bf16)  # [keys, chunk, 2*64]

        # ---- weights (pre-permuted host-side: [p, which, c, e], contiguous)
        w_sb = singles.tile([128, 3, CCH, 128], bf16)
        nc.sync.dma_start(out=w_sb, in_=wqkv[:, :, :, :])
        xq_r = xq.rearrange("(c p) n -> p c n", p=128)
        xk_r = xk.rearrange("(c p) n -> p c n", p=128)
        xq_c = [singles.tile([128, N], bf16, name=f"xq_c{c}") for c in range(CCH)]
        xk_c = [singles.tile([128, N], bf16, name=f"xk_c{c}") for c in range(CCH)]

        for nq in (slice(0, 512), slice(512, 1024), slice(1024, 2048),
                   slice(2048, 4096)):
            for c in range(CCH):
                nc.sync.dma_start(out=xk_c[c][:, nq], in_=xk_r[:, c, nq])
                nc.scalar.dma_start(out=xq_c[c][:, nq], in_=xq_r[:, c, nq])

        # PE HAM pre-warm during the input-DMA wait: ~30 tiny matmuls keep
        # the PE busy so the clock gate opens to 8/8 before projections.
        warm_ps = stps.tile([16, 8], f32, tag="st", name="warm_ps")
        for i in range(30):
            nc.tensor.matmul(
                warm_ps[:, :], lhsT=warm_sb[:, :], rhs=warm_sb[:, 0:8],
                start=True, stop=True, skip_group_check=True,
            )

        def emit_proj(t):
            """Projections for tile t: qt/kt slices + v chunks."""
            ns = slice(512 * t, 512 * t + 512)
            qt_ps = stps.tile([128, 512], f32, tag="st", name="qt_ps")
            for c in range(CCH):
                nc.tensor.matmul(
                    qt_ps[:, :], lhsT=w_sb[:, 0, c, :], rhs=xq_c[c][:, ns],
                    start=(c == 0), stop=(c == CCH - 1),
                )
            nc.scalar.activation(qt_all[:, ns], qt_ps[:, :], Ident, bias=bq_sb)
            kt_ps = stps.tile([128, 512], f32, tag="st", name="kt_ps")
            for c in range(CCH):
                nc.tensor.matmul(
                    kt_ps[:, :], lhsT=w_sb[:, 1, c, :], rhs=xk_c[c][:, ns],
                    start=(c == 0), stop=(c == CCH - 1),
                )
            nc.scalar.activation(kt_all[:, ns], kt_ps[:, :], Ident, bias=bk_sb)
            for jj in range(4):
                i = 4 * t + jj
                nsj = slice(512 * t + 128 * jj, 512 * t + 128 * jj + 128)
                v_ps = stps.tile([128, 128], f32, tag="st", name="v_ps")
                for c in range(CCH):
                    nc.tensor.matmul(
                        v_ps[:, :],
                        lhsT=xk_c[c][:, nsj],
                        rhs=w_sb[:, 2, c, :],
                        start=(c == 0), stop=(c == CCH - 1),
                    )
                nc.vector.tensor_add(v_all[:, i, :], v_ps[:, :], bvb_sb)

        def emit_attn(t, emit_next_proj):
            """Causal attention for tile t (lookahead-2 pipelined emission)."""
            ns = slice(512 * t, 512 * t + 512)
            nki = 4 * t + 4
            # group list: off-diagonal pairs then diagonal pairs
            # each entry: (chunks, is_diag) with chunks a list of (ki, col_off)
            groups = []
            for g in range(2 * t):
                groups.append(([(2 * g, 0), (2 * g + 1, 0)], False))
            for d in range(2):
                groups.append(
                    ([(4 * t + r, 128 * r) for r in (2 * d, 2 * d + 1)], True)
                )
            ngrp = len(groups)

            av_ps = avps.tile([128, 512], f32, tag="av", name="av_ps")
            den_ps = denps.tile([128, 512], f32, tag="den", name="den_ps")
            # NOTE: start=True clears has_written only for the REGION the
            # matmul writes (not the whole bank), so every distinct output
            # region (column-tile) needs its own start=True per tile.
            state = {}

            def emit_scores(gi):
                chunks, _ = groups[gi]
                sts = {}
                for h in range(2):
                    hp = slice(64 * h, 64 * h + 64)
                    st = stps.tile([128, 2, 512], f32, tag="st", name=f"st{h}")
                    for i, (ki, off) in enumerate(chunks):
                        nc.tensor.matmul(
                            st[:, i, off:512],
                            lhsT=kt_all[hp, 128 * ki : 128 * ki + 128],
                            rhs=qt_all[hp, 512 * t + off : 512 * t + 512],
                            start=True, stop=True,
                            tile_position=(64 * h, 0),
                        )
                    sts[h] = st
                return sts

            def emit_p(gi, sts):
                chunks, is_diag = groups[gi]
                ps = {}
                for h in range(2):
                    st = sts[h]
                    p_sb = ppool.tile(
                        [128, 2, 512], bf16, tag=f"p{h}", name=f"p{h}", bufs=4
                    )
                    if is_diag:
                        # ACT exact exp, one rectangular instruction per pair
                        # (the [off0, off1) strip of the second chunk is
                        # stale-psum garbage that nothing downstream reads)
                        off0 = chunks[0][1]
                        nc.scalar.activation(
                            p_sb[:, :, off0:512], st[:, :, off0:512],
                            Exp, scale=LN2 / 128.0,
                        )
                        # causal mask on the [off, off+128) block
                        for i, (ki, off) in enumerate(chunks):
                            nc.vector.tensor_mul(
                                p_sb[:, i, off : off + 128],
                                p_sb[:, i, off : off + 128],
                                mask_sb[:, :],
                            )
                    elif h == 0:
                        nc.scalar.activation(
                            p_sb[:, :, :], st[:, :, :], Exp, scale=LN2 / 128.0
                        )
                    else:
                        # DVE Schraudolph: bf16 bits = int16(st + C2)
                        nc.vector.tensor_scalar(
                            p_sb[:, :, :].bitcast(i16), st[:, :, :],
                            C2, None, Add,
                        )
                    ps[h] = p_sb
                return ps

            def emit_av(gi, ps):
                chunks, _ = groups[gi]
                last_gi = gi == ngrp - 1
                # den slot first: 4 concurrent M=32 ones-broadcast matmuls
                # (tile 0 has no off-diagonal groups, so rows 64..127 would
                # stay unwritten; fold both chunk-parities onto cols 0/32).
                for i, (ki, off) in enumerate(chunks):
                    for h in range(2):
                        cp = 32 * (2 * i + h) if t > 0 else 32 * h
                        nc.tensor.matmul(
                            den_ps[cp : cp + 32, off:512],
                            lhsT=ones32[:, :],
                            rhs=ps[h][:, i, off:512],
                            start=not state.get(f"den{cp}", False),
                            stop=(last_gi and i == len(chunks) - 1 and h == 1),
                            tile_position=(0, cp),
                            skip_group_check=True,
                        )
                        state[f"den{cp}"] = True
                # AV: two heads column-tiled, concurrent per chunk
                for i, (ki, off) in enumerate(chunks):
                    for h in range(2):
                        nc.tensor.matmul(
                            av_ps[64 * h : 64 * h + 64, off:512],
                            lhsT=v_all[:, ki, 64 * h : 64 * h + 64],
                            rhs=ps[h][:, i, off:512],
                            start=not state.get(f"av{h}", False),
                            stop=(last_gi and i == len(chunks) - 1 and h == 1),
                            tile_position=(0, 64 * h),
                            skip_group_check=True,
                        )
                        state[f"av{h}"] = True

            # lookahead-2 pipeline; projections for the next tile slot in
            # right before the final AV batch (keeps PE fed while the last
            # groups' exp/ts complete).
            sts = {0: emit_scores(0)}
            if ngrp > 1:
                sts[1] = emit_scores(1)
            for gi in range(ngrp):
                cur_p = emit_p(gi, sts.pop(gi))
                if gi + 2 < ngrp:
                    sts[gi + 2] = emit_scores(gi + 2)
                if gi == ngrp - 1:
                    emit_next_proj()
                emit_av(gi, cur_p)

            # ---- finalize: evacuate AV (ACT), fold den quadrants, DMA-pack
            # the two den rows to [128,8], exact reciprocal, DMA-broadcast
            # back, one [128,512] normalize mul, one output DMA.
            av_sb = opool.tile([128, 512], f32, tag="avsb", name="av_sb")
            nc.scalar.activation(av_sb[:, :], av_ps[:, :], Ident)
            denf = opool.tile([64, 512], f32, tag="denf", name="denf")
            if t == 0:
                nc.vector.tensor_copy(denf[:, :], den_ps[0:64, :])
            else:
                den_hi = opool.tile([64, 512], f32, tag="denhi", name="den_hi")
                nc.scalar.activation(den_hi[:, :], den_ps[64:128, :], Ident)
                nc.vector.tensor_add(denf[:, :], den_ps[0:64, :], den_hi[:, :])
            # rows 0..31 of denf are 32 copies of den_h0, rows 32..63 den_h1
            dd = drs.tile([2, 512], f32, tag="dd", name="dd")
            for h in range(2):
                nc.gpsimd.dma_start(
                    out=dd[h : h + 1, :], in_=denf[32 * h : 32 * h + 1, :]
                )
            packed = opool.tile([128, 8], f32, tag="packed", name="packed")
            nc.gpsimd.dma_start(
                out=packed[:, :], in_=dd.rearrange("h (p x) -> (h p) x", p=64)
            )
            r_sb = opool.tile([128, 8], f32, tag="r_sb", name="r_sb")
            nc.vector.reciprocal(r_sb[:, :], packed[:, :])
            dd2 = drs.tile([2, 512], f32, tag="dd2", name="dd2")
            nc.gpsimd.dma_start(
                out=dd2.rearrange("h (p x) -> (h p) x", p=64), in_=r_sb[:, :]
            )
            rb = opool.tile([128, 512], f32, tag="rb", name="rb")
            for h in range(2):
                nc.gpsimd.dma_start(
                    out=rb[64 * h : 64 * h + 64, :],
                    in_=dd2[h : h + 1, :].to_broadcast([64, 512]),
                )
            out_t = opool.tile([128, 512], f32, tag="out", name="out_t")
            nc.vector.tensor_mul(out_t[:, :], av_sb[:, :], rb[:, :])
            nc.sync.dma_start(out=o[:, ns], in_=out_t[:, :])

        # ================= pipelined main loop =================
        emit_proj(0)
        for t in range(NT):
            nxt = (lambda tt=t + 1: emit_proj(tt)) if t + 1 < NT else (lambda: None)
            emit_attn(t, nxt)

    _split_multi_waits(nc, mybir, bass_rust)
    return nc


def kernel(query, key, Wq, bq, Wk, bk, Wv, bv):
    from concourse.bass_utils import run_bass_kernel_spmd

    global last_results
    if "nc" not in _cache:
        _cache["nc"] = _build_program()
    nc = _cache["nc"]

    query = np.asarray(query, np.float32)
    key = np.asarray(key, np.float32)
    Wq = np.asarray(Wq, np.float32)
    Wk = np.asarray(Wk, np.float32)
    Wv = np.asarray(Wv, np.float32)
    bq = np.asarray(bq, np.float32)
    bk = np.asarray(bk, np.float32)
    bv = np.asarray(bv, np.float32)

    # shared per-batch inputs
    xq_b = [query[b].reshape(C, N).astype(ml_dtypes.bfloat16) for b in range(B)]
    xk_b = [key[b].reshape(C, N).astype(ml_dtypes.bfloat16) for b in range(B)]

    # causal mask for the diagonal 128-block: mask[kk, qq] = qq >= kk
    kk = np.arange(128)[:, None]
    qq = np.arange(128)[None, :]
    mask = (qq >= kk).astype(ml_dtypes.bfloat16)

    qscale = C1 / 8.0  # 1/sqrt(hd) plus the exp prescale

    in_maps = []
    for core in range(8):
        b, p = core // 4, core % 4
        sl = slice(128 * p, 128 * p + 128)
        # [C, e] -> [c, p, e] -> [p, c, e] (contiguous per partition)
        wq_h = (Wq[sl] * qscale).T.reshape(CCH, 128, 128).transpose(1, 0, 2)
        wk_h = Wk[sl].T.reshape(CCH, 128, 128).transpose(1, 0, 2)
        wv_h = Wv[sl].T.reshape(CCH, 128, 128).transpose(1, 0, 2)
        wqkv = np.ascontiguousarray(
            np.stack([wq_h, wk_h, wv_h], axis=1)
        ).astype(ml_dtypes.bfloat16)
        smalls = np.ascontiguousarray(
            np.concatenate(
                [
                    (bq[sl] * qscale)[:, None],
                    bk[sl][:, None],
                    np.broadcast_to(bv[sl], (128, 128)),
                ],
                axis=1,
            )
        ).astype(np.float32)
        in_maps.append(
            {
                "xq": xq_b[b],
                "xk": xk_b[b],
                "wqkv": wqkv,
                "smalls": smalls,
                "mask": mask,
            }
        )

    trace = bool(int(os.environ.get("KERNEL_TRACE", "0")))
    res = run_bass_kernel_spmd(nc, in_maps, core_ids=list(range(8)), trace=trace)
    last_results = res

    out = np.empty((B, E, H, W), np.float32)
    for core in range(8):
        b, p = core // 4, core % 4
        out[b, 128 * p : 128 * p + 128] = res.results[core]["o"].reshape(128, H, W)
    return out


# revision 11
# speedup vs baseline: 1.1505x; 1.1505x over previous
"""CausalAttention2d Trainium2 kernel (8-core SPMD), v4.

Shards (B=2, heads=8) -> 16 (batch, head) pairs across 8 cores: core c handles
batch b=c//4 and head-pair p=c%4 (heads 2p, 2p+1 = E-channels 128p..128p+128).
Each core computes Q/K/V projections for its head pair, causal softmax
attention over the full N=4096 sequence, and writes its [128, 4096] slice of
the channel-major output. Host assembles the full (2, 512, 64, 64) tensor.

v4 changes over v3 (188us):
 - AV matmuls column-tiled: the two heads' AV run CONCURRENTLY on PE column
   halves (tile_position (0,0)/(0,64), M=64 each) into one [128,512] psum
   bank -> 216ns per key-chunk for BOTH heads (was 432, HW-verified).
 - Softmax denominators come from a dedicated den-slot: per group, four
   concurrent M=32 ones-broadcast matmuls (col positions 0/32/64/96) that
   accumulate 32 identical copies of each (head, chunk-parity) partial den
   into a second psum bank; one ACT cross-base evac + one DVE add folds the
   quadrants. Den slot runs BEFORE the AV slots so the last tile's
   denominator roundtrip overlaps its final AV matmuls.
 - Finalize: ACT (not DVE) evacuates AV psum; ONE [128,512] DVE mul
   normalizes both heads; one DMA per tile writes the output.
 - Input staging: biases/mask merged into single contiguous staging tensors
   and weights sent pre-permuted, killing the per-partition 4-byte DMA
   descriptor storm that delayed the first matmul to 14.5us.
 - ~30 tiny warm-up matmuls during the input-DMA wait pre-warm the PE HAM
   clock gate (cold K=4/8 -> warm K=8/8).
 - exp split tuned: head0 exact on ACT; head1 Schraudolph on DVE for cols
   [0:SPL2) of each 512-chunk, exact ACT tail for the rest (engine balance).
"""

import os

import numpy as np
import ml_dtypes

B, C, H, W = 2, 512, 64, 64
N = H * W          # 4096
E = 512
NH = 8
HD = 64
NT = 8             # n-tiles of 512
CCH = 4            # contraction chunks of 128 over C

LN2 = 0.6931471805599453
C1 = 128.0 / LN2           # score prescale (folded into Wq host-side)
C2 = 16248.75              # Schraudolph bias (calibrated, robust to rounding mode)
SPL2 = 480                 # head1 exp: DVE Schraudolph cols [0:SPL2), ACT tail

_cache = {}
last_results = None  # BassKernelResults of the most recent run (for profiling)


def _split_multi_waits(nc, mybir, bass_rust):
    """This walrus build accepts only ONE sync-wait per instruction; hoist
    extra waits onto single-wait NOPs inserted just before, preserving
    per-engine program order."""
    n = 0
    for f in nc.m.functions:
        for bb in f.blocks:
            old = list(bb.instructions)
            new = []
            changed = False
            for inst in old:
                si = inst.sync_info
                if si is not None and si.on_wait and len(si.on_wait) > 1:
                    waits = list(si.on_wait)
                    for w in waits[:-1]:
                        nop = bass_rust.InstNoOp(
                            name=f"I-sw{n}", engine=inst.engine, ins=[], outs=[]
                        )
                        n += 1
                        nop.sync_info = mybir.SyncInfo(on_wait=[w], on_update=[])
                        new.append(nop)
                    inst.sync_info = mybir.SyncInfo(
                        on_wait=[waits[-1]], on_update=list(si.on_update)
                    )
                    changed = True
                new.append(inst)
            if changed:
                bb.instructions = new
    return n


def _build_program():
    import bass_rust
    import concourse.bass as bass
    import concourse.mybir as mybir
    import concourse.tile as tile
    from contextlib import ExitStack

    f32 = mybir.dt.float32
    bf16 = mybir.dt.bfloat16
    i16 = mybir.dt.int16
    Exp = mybir.ActivationFunctionType.Exp
    Ident = mybir.ActivationFunctionType.Identity
    Add = mybir.AluOpType.add

    nc = bass.Bass()
    xq = nc.dram_tensor("xq", [C, N], bf16, kind="ExternalInput")
    xk = nc.dram_tensor("xk", [C, N], bf16, kind="ExternalInput")
    wqkv = nc.dram_tensor("wqkv", [128, 3, CCH, 128], bf16, kind="ExternalInput")
    smalls = nc.dram_tensor("smalls", [128, 130], f32, kind="ExternalInput")
    mask = nc.dram_tensor("mask", [128, 128], bf16, kind="ExternalInput")
    o = nc.dram_tensor("o", [128, N], f32, kind="ExternalOutput")

    with tile.TileContext(nc) as tc, ExitStack() as ctx:
        singles = ctx.enter_context(tc.tile_pool(name="singles", bufs=1))
        ppool = ctx.enter_context(tc.tile_pool(name="ppool", bufs=2))
        opool = ctx.enter_context(tc.tile_pool(name="opool", bufs=3))
        drs = ctx.enter_context(tc.tile_pool(name="drs", bufs=2, space="DRAM"))
        # PSUM budget (8 banks): st ring 3 bufs x 2 banks (scores AND
        # projection psums share it) + av 1 bank + den 1 bank.
        stps = ctx.enter_context(tc.tile_pool(name="stps", bufs=3, space="PSUM"))
        avps = ctx.enter_context(tc.tile_pool(name="avps", bufs=1, space="PSUM"))
        denps = ctx.enter_context(tc.tile_pool(name="denps", bufs=1, space="PSUM"))

        smalls_sb = singles.tile([128, 130], f32)
        mask_sb = singles.tile([128, 128], bf16)
        ones32 = singles.tile([128, 32], bf16)
        warm_sb = singles.tile([128, 16], bf16)
        # warm-up: force the exp ACT table set resident before the first
        # real activation (overlaps the input DMAs instead of serializing
        # into the first tile's dependency chain)
        warm = singles.tile([1, 8], f32)
        nc.vector.memset(warm, 0.0)
        nc.vector.memset(ones32, 1.0)
        nc.vector.memset(warm_sb, 1.0)
        nc.scalar.activation(warm[:, :], warm[:, :], Exp)
        nc.sync.dma_start(out=smalls_sb, in_=smalls[:, :])
        nc.sync.dma_start(out=mask_sb, in_=mask[:, :])
        bq_sb = smalls_sb[:, 0:1]
        bk_sb = smalls_sb[:, 1:2]
        bvb_sb = smalls_sb[:, 2:130]

        # persistent activations
        qt_all = singles.tile([128, N], bf16)   # [2*64 d, q] (prescaled by C1)
        kt_all = singles.tile([128, N], bf16)   # [2*64 d, keys]
        v_all = singles.tile([128, N // 128, 128], bf16)  # [keys, chunk, 2*64]

        # ---- weights (pre-permuted host-side: [p, which, c, e], contiguous)
        w_sb = singles.tile([128, 3, CCH, 128], bf16)
        nc.sync.dma_start(out=w_sb, in_=wqkv[:, :, :, :])
        xq_r = xq.rearrange("(c p) n -> p c n", p=128)
        xk_r = xk.rearrange("(c p) n -> p c n", p=128)
        xq_c = [singles.tile([128, N], bf16, name=f"xq_c{c}") for c in range(CCH)]
        xk_c = [singles.tile([128, N], bf16, name=f"xk_c{c}") for c in range(CCH)]

        for nq in (slice(0, 512), slice(512, 1024), slice(1024, 2048),
                   slice(2048, 4096)):
            for c in range(CCH):
                nc.sync.dma_start(out=xk_c[c][:, nq], in_=xk_r[:, c, nq])
                nc.gpsimd.dma_start(out=xq_c[c][:, nq], in_=xq_r[:, c, nq])

        # PE HAM pre-warm during the input-DMA wait: ~30 tiny matmuls keep
        # the PE busy so the clock gate opens to 8/8 before projections.
        warm_ps = stps.tile([16, 8], f32, tag="st", name="warm_ps")
        for i in range(30):
            nc.tensor.matmul(
                warm_ps[:, :], lhsT=warm_sb[:, :], rhs=warm_sb[:, 0:8],
                start=True, stop=True, skip_group_check=True,
            )

        def emit_proj(t):
            """Projections for tile t: qt/kt slices + v chunks."""
            ns = slice(512 * t, 512 * t + 512)
            qt_ps = stps.tile([128, 512], f32, tag="st", name="qt_ps")
            for c in range(CCH):
                nc.tensor.matmul(
                    qt_ps[:, :], lhsT=w_sb[:, 0, c, :], rhs=xq_c[c][:, ns],
                    start=(c == 0), stop=(c == CCH - 1),
                )
            nc.vector.tensor_scalar(qt_all[:, ns], qt_ps[:, :], bq_sb, None, Add)
            kt_ps = stps.tile([128, 512], f32, tag="st", name="kt_ps")
            for c in range(CCH):
                nc.tensor.matmul(
                    kt_ps[:, :], lhsT=w_sb[:, 1, c, :], rhs=xk_c[c][:, ns],
                    start=(c == 0), stop=(c == CCH - 1),
                )
            nc.vector.tensor_scalar(kt_all[:, ns], kt_ps[:, :], bk_sb, None, Add)
            for jj in range(4):
                i = 4 * t + jj
                nsj = slice(512 * t + 128 * jj, 512 * t + 128 * jj + 128)
                v_ps = stps.tile([128, 128], f32, tag="st", name="v_ps")
                for c in range(CCH):
                    nc.tensor.matmul(
                        v_ps[:, :],
                        lhsT=xk_c[c][:, nsj],
                        rhs=w_sb[:, 2, c, :],
                        start=(c == 0), stop=(c == CCH - 1),
                    )
                nc.vector.tensor_add(v_all[:, i, :], v_ps[:, :], bvb_sb)

        def emit_attn(t, emit_next_proj):
            """Causal attention for tile t (lookahead-2 pipelined emission)."""
            ns = slice(512 * t, 512 * t + 512)
            nki = 4 * t + 4
            # group list: off-diagonal pairs then diagonal pairs
            # each entry: (chunks, is_diag) with chunks a list of (ki, col_off)
            groups = []
            for g in range(2 * t):
                groups.append(([(2 * g, 0), (2 * g + 1, 0)], False))
            for d in range(2):
                groups.append(
                    ([(4 * t + r, 128 * r) for r in (2 * d, 2 * d + 1)], True)
                )
            ngrp = len(groups)

            av_ps = avps.tile([128, 512], f32, tag="av", name="av_ps")
            den_ps = denps.tile([128, 512], f32, tag="den", name="den_ps")
            # NOTE: start=True clears has_written only for the REGION the
            # matmul writes (not the whole bank), so every distinct output
            # region (column-tile) needs its own start=True per tile.
            state = {}

            def emit_scores(gi):
                chunks, _ = groups[gi]
                sts = {}
                for h in range(2):
                    hp = slice(64 * h, 64 * h + 64)
                    st = stps.tile([128, 2, 512], f32, tag="st", name=f"st{h}")
                    for i, (ki, off) in enumerate(chunks):
                        nc.tensor.matmul(
                            st[:, i, off:512],
                            lhsT=kt_all[hp, 128 * ki : 128 * ki + 128],
                            rhs=qt_all[hp, 512 * t + off : 512 * t + 512],
                            start=True, stop=True,
                            tile_position=(64 * h, 0),
                        )
                    sts[h] = st
                return sts

            def emit_p(gi, sts):
                chunks, is_diag = groups[gi]
                ps = {}
                for h in range(2):
                    st = sts[h]
                    p_sb = ppool.tile(
                        [128, 2, 512], bf16, tag=f"p{h}", name=f"p{h}", bufs=4
                    )
                    if is_diag:
                        # exp over one rectangle per pair (the [off0, off1)
                        # strip of the second chunk is stale-psum garbage
                        # that nothing downstream reads); h0 exact on ACT,
                        # h1 Schraudolph on DVE (keeps the boundary-critical
                        # ACT queue short so next-tile evacs clear fast)
                        off0 = chunks[0][1]
                        if h == 0:
                            nc.scalar.activation(
                                p_sb[:, :, off0:512], st[:, :, off0:512],
                                Exp, scale=LN2 / 128.0,
                            )
                        else:
                            nc.vector.tensor_scalar(
                                p_sb[:, :, off0:512].bitcast(i16),
                                st[:, :, off0:512], C2, None, Add,
                            )
                        # causal mask on the [off, off+128) block
                        for i, (ki, off) in enumerate(chunks):
                            nc.vector.tensor_mul(
                                p_sb[:, i, off : off + 128],
                                p_sb[:, i, off : off + 128],
                                mask_sb[:, :],
                            )
                    elif h == 0:
                        nc.scalar.activation(
                            p_sb[:, :, :], st[:, :, :], Exp, scale=LN2 / 128.0
                        )
                    else:
                        # DVE Schraudolph: bf16 bits = int16(st + C2)
                        nc.vector.tensor_scalar(
                            p_sb[:, :, :].bitcast(i16), st[:, :, :],
                            C2, None, Add,
                        )
                    ps[h] = p_sb
                return ps

            def emit_av(gi, ps):
                chunks, _ = groups[gi]
                last_gi = gi == ngrp - 1
                # den slot first: 4 concurrent M=32 ones-broadcast matmuls
                # (tile 0 has no off-diagonal groups, so rows 64..127 would
                # stay unwritten; fold both chunk-parities onto cols 0/32).
                for i, (ki, off) in enumerate(chunks):
                    for h in range(2):
                        cp = 32 * (2 * i + h) if t > 0 else 32 * h
                        nc.tensor.matmul(
                            den_ps[cp : cp + 32, off:512],
                            lhsT=ones32[:, :],
                            rhs=ps[h][:, i, off:512],
                            start=not state.get(f"den{cp}", False),
                            stop=(last_gi and i == len(chunks) - 1 and h == 1),
                            tile_position=(0, cp),
                            skip_group_check=True,
                        )
                        state[f"den{cp}"] = True
                # AV: two heads column-tiled, concurrent per chunk
                for i, (ki, off) in enumerate(chunks):
                    for h in range(2):
                        nc.tensor.matmul(
                            av_ps[64 * h : 64 * h + 64, off:512],
                            lhsT=v_all[:, ki, 64 * h : 64 * h + 64],
                            rhs=ps[h][:, i, off:512],
                            start=not state.get(f"av{h}", False),
                            stop=(last_gi and i == len(chunks) - 1 and h == 1),
                            tile_position=(0, 64 * h),
                            skip_group_check=True,
                        )
                        state[f"av{h}"] = True

            # lookahead-2 pipeline; projections for the next tile slot in
            # right before the final AV batch (keeps PE fed while the last
            # groups' exp/ts complete).
            sts = {0: emit_scores(0)}
            if ngrp > 1:
                sts[1] = emit_scores(1)
            # emit next tile's projections BEFORE the diag groups so the
            # qt/kt evacuations clear the engine queues early and the next
            # tile's scores don't stall at the boundary
            proj_at = max(0, ngrp - 3)
            for gi in range(ngrp):
                cur_p = emit_p(gi, sts.pop(gi))
                if gi + 2 < ngrp:
                    sts[gi + 2] = emit_scores(gi + 2)
                if gi == proj_at:
                    emit_next_proj()
                emit_av(gi, cur_p)

            # ---- finalize: evacuate AV (ACT), fold den quadrants, DMA-pack
            # the two den rows to [128,8], exact reciprocal, DMA-broadcast
            # back, one [128,512] normalize mul, one output DMA.
            av_sb = opool.tile([128, 512], f32, tag="avsb", name="av_sb")
            nc.scalar.activation(av_sb[:, :], av_ps[:, :], Ident)
            denf = opool.tile([64, 512], f32, tag="denf", name="denf")
            if t == 0:
                nc.vector.tensor_copy(denf[:, :], den_ps[0:64, :])
            else:
                den_hi = opool.tile([64, 512], f32, tag="denhi", name="den_hi")
                nc.scalar.activation(den_hi[:, :], den_ps[64:128, :], Ident)
                nc.vector.tensor_add(denf[:, :], den_ps[0:64, :], den_hi[:, :])
            # rows 0..31 of denf are 32 copies of den_h0, rows 32..63 den_h1
            dd = drs.tile([2, 512], f32, tag="dd", name="dd")
            for h in range(2):
                nc.gpsimd.dma_start(
                    out=dd[h : h + 1, :], in_=denf[32 * h : 32 * h + 1, :]
                )
            packed = opool.tile([128, 8], f32, tag="packed", name="packed")
            nc.gpsimd.dma_start(
                out=packed[:, :], in_=dd.rearrange("h (p x) -> (h p) x", p=64)
            )
            r_sb = opool.tile([128, 8], f32, tag="r_sb", name="r_sb")
            nc.vector.reciprocal(r_sb[:, :], packed[:, :])
            dd2 = drs.tile([2, 512], f32, tag="dd2", name="dd2")
            nc.gpsimd.dma_start(
                out=dd2.rearrange("h (p x) -> (h p) x", p=64), in_=r_sb[:, :]
            )
            rb = opool.tile([128, 512], f32, tag="rb", name="rb")
            for h in range(2):
                nc.gpsimd.dma_start(
                    out=rb[64 * h : 64 * h + 64, :],
                    in_=dd2[h : h + 1, :].to_broadcast([64, 512]),
                )
            out_t = opool.tile([128, 512], f32, tag="out", name="out_t")
            nc.vector.tensor_mul(out_t[:, :], av_sb[:, :], rb[:, :])
            nc.sync.dma_start(out=o[:, ns], in_=out_t[:, :])

        # ================= pipelined main loop =================
        emit_proj(0)
        for t in range(NT):
            nxt = (lambda tt=t + 1: emit_proj(tt)) if t + 1 < NT else (lambda: None)
            emit_attn(t, nxt)

    _split_multi_waits(nc, mybir, bass_rust)
    return nc


def kernel(query, key, Wq, bq, Wk, bk, Wv, bv):
    from concourse.bass_utils import run_bass_kernel_spmd

    global last_results
    if "nc" not in _cache:
        _cache["nc"] = _build_program()
    nc = _cache["nc"]

    query = np.asarray(query, np.float32)
    key = np.asarray(key, np.float32)
    Wq = np.asarray(Wq, np.float32)
    Wk = np.asarray(Wk, np.float32)
    Wv = np.asarray(Wv, np.float32)
    bq = np.asarray(bq, np.float32)
    bk = np.asarray(bk, np.float32)
    bv = np.asarray(bv, np.float32)

    # shared per-batch inputs
    xq_b = [query[b].reshape(C, N).astype(ml_dtypes.bfloat16) for b in range(B)]
    xk_b = [key[b].reshape(C, N).astype(ml_dtypes.bfloat16) for b in range(B)]

    # causal mask for the diagonal 128-block: mask[kk, qq] = qq >= kk
    kk = np.arange(128)[:, None]
    qq = np.arange(128)[None, :]
    mask = (qq >= kk).astype(ml_dtypes.bfloat16)

    qscale = C1 / 8.0  # 1/sqrt(hd) plus the exp prescale

    in_maps = []
    for core in range(8):
        b, p = core // 4, core % 4
        sl = slice(128 * p, 128 * p + 128)
        # [C, e] -> [c, p, e] -> [p, c, e] (contiguous per partition)
        wq_h = (Wq[sl] * qscale).T.reshape(CCH, 128, 128).transpose(1, 0, 2)
        wk_h = Wk[sl].T.reshape(CCH, 128, 128).transpose(1, 0, 2)
        wv_h = Wv[sl].T.reshape(CCH, 128, 128).transpose(1, 0, 2)
        wqkv = np.ascontiguousarray(
            np.stack([wq_h, wk_h, wv_h], axis=1)
        ).astype(ml_dtypes.bfloat16)
        smalls = np.ascontiguousarray(
            np.concatenate(
                [
                    (bq[sl] * qscale)[:, None],
                    bk[sl][:, None],
                    np.broadcast_to(bv[sl], (128, 128)),
                ],
                axis=1,
            )
        ).astype(np.float32)
        in_maps.append(
            {
                "xq": xq_b[b],
                "xk": xk_b[b],
                "wqkv": wqkv,
                "smalls": smalls,
                "mask": mask,
            }
        )

    trace = bool(int(os.environ.get("KERNEL_TRACE", "0")))
    res = run_bass_kernel_spmd(nc, in_maps, core_ids=list(range(8)), trace=trace)
    last_results = res

    out = np.empty((B, E, H, W), np.float32)
    for core in range(8):
        b, p = core // 4, core % 4
        out[b, 128 * p : 128 * p + 128] = res.results[core]["o"].reshape(128, H, W)
    return out


# revision 13
# speedup vs baseline: 1.1823x; 1.0277x over previous
"""CausalAttention2d Trainium2 kernel (8-core SPMD), v4.

Shards (B=2, heads=8) -> 16 (batch, head) pairs across 8 cores: core c handles
batch b=c//4 and head-pair p=c%4 (heads 2p, 2p+1 = E-channels 128p..128p+128).
Each core computes Q/K/V projections for its head pair, causal softmax
attention over the full N=4096 sequence, and writes its [128, 4096] slice of
the channel-major output. Host assembles the full (2, 512, 64, 64) tensor.

v4 changes over v3 (188us):
 - AV matmuls column-tiled: the two heads' AV run CONCURRENTLY on PE column
   halves (tile_position (0,0)/(0,64), M=64 each) into one [128,512] psum
   bank -> 216ns per key-chunk for BOTH heads (was 432, HW-verified).
 - Softmax denominators come from a dedicated den-slot: per group, four
   concurrent M=32 ones-broadcast matmuls (col positions 0/32/64/96) that
   accumulate 32 identical copies of each (head, chunk-parity) partial den
   into a second psum bank; one ACT cross-base evac + one DVE add folds the
   quadrants. Den slot runs BEFORE the AV slots so the last tile's
   denominator roundtrip overlaps its final AV matmuls.
 - Finalize: ACT (not DVE) evacuates AV psum; ONE [128,512] DVE mul
   normalizes both heads; one DMA per tile writes the output.
 - Input staging: biases/mask merged into single contiguous staging tensors
   and weights sent pre-permuted, killing the per-partition 4-byte DMA
   descriptor storm that delayed the first matmul to 14.5us.
 - ~30 tiny warm-up matmuls during the input-DMA wait pre-warm the PE HAM
   clock gate (cold K=4/8 -> warm K=8/8).
 - exp split tuned: head0 exact on ACT; head1 Schraudolph on DVE for cols
   [0:SPL2) of each 512-chunk, exact ACT tail for the rest (engine balance).
"""

import os

import numpy as np
import ml_dtypes

B, C, H, W = 2, 512, 64, 64
N = H * W          # 4096
E = 512
NH = 8
HD = 64
NT = 8             # n-tiles of 512
CCH = 4            # contraction chunks of 128 over C

LN2 = 0.6931471805599453
C1 = 128.0 / LN2           # score prescale (folded into Wq host-side)
C2 = 16248.75              # Schraudolph bias (calibrated, robust to rounding mode)
SPL2 = 480                 # head1 exp: DVE Schraudolph cols [0:SPL2), ACT tail

_cache = {}
last_results = None  # BassKernelResults of the most recent run (for profiling)


def _split_multi_waits(nc, mybir, bass_rust):
    """This walrus build accepts only ONE sync-wait per instruction; hoist
    extra waits onto single-wait NOPs inserted just before, preserving
    per-engine program order."""
    n = 0
    for f in nc.m.functions:
        for bb in f.blocks:
            old = list(bb.instructions)
            new = []
            changed = False
            for inst in old:
                si = inst.sync_info
                if si is not None and si.on_wait and len(si.on_wait) > 1:
                    waits = list(si.on_wait)
                    for w in waits[:-1]:
                        nop = bass_rust.InstNoOp(
                            name=f"I-sw{n}", engine=inst.engine, ins=[], outs=[]
                        )
                        n += 1
                        nop.sync_info = mybir.SyncInfo(on_wait=[w], on_update=[])
                        new.append(nop)
                    inst.sync_info = mybir.SyncInfo(
                        on_wait=[waits[-1]], on_update=list(si.on_update)
                    )
                    changed = True
                new.append(inst)
            if changed:
                bb.instructions = new
    return n


def _build_program():
    import bass_rust
    import concourse.bass as bass
    import concourse.mybir as mybir
    import concourse.tile as tile
    from contextlib import ExitStack

    f32 = mybir.dt.float32
    bf16 = mybir.dt.bfloat16
    i16 = mybir.dt.int16
    Exp = mybir.ActivationFunctionType.Exp
    Ident = mybir.ActivationFunctionType.Identity
    Add = mybir.AluOpType.add

    nc = bass.Bass()
    xq = nc.dram_tensor("xq", [C, N], bf16, kind="ExternalInput")
    xk = nc.dram_tensor("xk", [C, N], bf16, kind="ExternalInput")
    wqkv = nc.dram_tensor("wqkv", [128, 3, CCH, 128], bf16, kind="ExternalInput")
    smalls = nc.dram_tensor("smalls", [128, 130], f32, kind="ExternalInput")
    mask = nc.dram_tensor("mask", [128, 128], bf16, kind="ExternalInput")
    o = nc.dram_tensor("o", [128, N], f32, kind="ExternalOutput")

    with tile.TileContext(nc) as tc, ExitStack() as ctx:
        singles = ctx.enter_context(tc.tile_pool(name="singles", bufs=1))
        ppool = ctx.enter_context(tc.tile_pool(name="ppool", bufs=2))
        opool = ctx.enter_context(tc.tile_pool(name="opool", bufs=3))
        drs = ctx.enter_context(tc.tile_pool(name="drs", bufs=2, space="DRAM"))
        # PSUM budget (8 banks): st ring 3 bufs x 2 banks (scores AND
        # projection psums share it) + av 1 bank + den 1 bank.
        stps = ctx.enter_context(tc.tile_pool(name="stps", bufs=3, space="PSUM"))
        avps = ctx.enter_context(tc.tile_pool(name="avps", bufs=1, space="PSUM"))
        denps = ctx.enter_context(tc.tile_pool(name="denps", bufs=1, space="PSUM"))

        smalls_sb = singles.tile([128, 130], f32)
        mask_sb = singles.tile([128, 128], bf16)
        ones32 = singles.tile([128, 32], bf16)
        warm_sb = singles.tile([128, 16], bf16)
        # warm-up: force the exp ACT table set resident before the first
        # real activation (overlaps the input DMAs instead of serializing
        # into the first tile's dependency chain)
        warm = singles.tile([1, 8], f32)
        nc.vector.memset(warm, 0.0)
        nc.vector.memset(ones32, 1.0)
        nc.vector.memset(warm_sb, 1.0)
        nc.scalar.activation(warm[:, :], warm[:, :], Exp)
        nc.sync.dma_start(out=smalls_sb, in_=smalls[:, :])
        nc.sync.dma_start(out=mask_sb, in_=mask[:, :])
        bq_sb = smalls_sb[:, 0:1]
        bk_sb = smalls_sb[:, 1:2]
        bvb_sb = smalls_sb[:, 2:130]

        # persistent activations
        qt_all = singles.tile([128, N], bf16)   # [2*64 d, q] (prescaled by C1)
        kt_all = singles.tile([128, N], bf16)   # [2*64 d, keys]
        v_all = singles.tile([128, N // 128, 128], bf16)  # [keys, chunk, 2*64]

        # ---- weights (pre-permuted host-side: [p, which, c, e], contiguous)
        w_sb = singles.tile([128, 3, CCH, 128], bf16)
        nc.sync.dma_start(out=w_sb, in_=wqkv[:, :, :, :])
        xq_r = xq.rearrange("(c p) n -> p c n", p=128)
        xk_r = xk.rearrange("(c p) n -> p c n", p=128)
        xq_c = [singles.tile([128, N], bf16, name=f"xq_c{c}") for c in range(CCH)]
        xk_c = [singles.tile([128, N], bf16, name=f"xk_c{c}") for c in range(CCH)]

        for nq in (slice(0, 512), slice(512, 1024), slice(1024, 2048),
                   slice(2048, 4096)):
            for c in range(CCH):
                nc.sync.dma_start(out=xk_c[c][:, nq], in_=xk_r[:, c, nq])
                nc.gpsimd.dma_start(out=xq_c[c][:, nq], in_=xq_r[:, c, nq])

        # PE HAM pre-warm during the input-DMA wait: ~30 tiny matmuls keep
        # the PE busy so the clock gate opens to 8/8 before projections.
        warm_ps = stps.tile([16, 8], f32, tag="st", name="warm_ps")
        for i in range(30):
            nc.tensor.matmul(
                warm_ps[:, :], lhsT=warm_sb[:, :], rhs=warm_sb[:, 0:8],
                start=True, stop=True, skip_group_check=True,
            )

        def emit_proj(t):
            """Projections for tile t: qt/kt slices + v chunks."""
            ns = slice(512 * t, 512 * t + 512)
            qt_ps = stps.tile([128, 512], f32, tag="st", name="qt_ps")
            for c in range(CCH):
                nc.tensor.matmul(
                    qt_ps[:, :], lhsT=w_sb[:, 0, c, :], rhs=xq_c[c][:, ns],
                    start=(c == 0), stop=(c == CCH - 1),
                )
            nc.scalar.activation(qt_all[:, ns], qt_ps[:, :], Ident, bias=bq_sb)
            kt_ps = stps.tile([128, 512], f32, tag="st", name="kt_ps")
            for c in range(CCH):
                nc.tensor.matmul(
                    kt_ps[:, :], lhsT=w_sb[:, 1, c, :], rhs=xk_c[c][:, ns],
                    start=(c == 0), stop=(c == CCH - 1),
                )
            nc.vector.tensor_scalar(kt_all[:, ns], kt_ps[:, :], bk_sb, None, Add)
            for jj in range(4):
                i = 4 * t + jj
                nsj = slice(512 * t + 128 * jj, 512 * t + 128 * jj + 128)
                v_ps = stps.tile([128, 128], f32, tag="st", name="v_ps")
                for c in range(CCH):
                    nc.tensor.matmul(
                        v_ps[:, :],
                        lhsT=xk_c[c][:, nsj],
                        rhs=w_sb[:, 2, c, :],
                        start=(c == 0), stop=(c == CCH - 1),
                    )
                nc.vector.tensor_add(v_all[:, i, :], v_ps[:, :], bvb_sb)

        # ============== flat cross-tile group pipeline ==============
        # flat list of attention groups across ALL tiles; the lookahead-2
        # software pipeline runs over this list so the pipeline never breaks
        # at tile boundaries. Each entry: (t, gi, ngrp, chunks, is_diag)
        flat = []
        for t in range(NT):
            groups = []
            for g in range(2 * t):
                groups.append(([(2 * g, 0), (2 * g + 1, 0)], False))
            for d in range(2):
                groups.append(
                    ([(4 * t + r, 128 * r) for r in (2 * d, 2 * d + 1)], True)
                )
            for gi, (chunks, is_diag) in enumerate(groups):
                flat.append((t, gi, len(groups), chunks, is_diag))

        # per-tile psum contexts, created lazily
        # NOTE: start=True clears has_written only for the REGION the matmul
        # writes (not the whole bank), so every distinct output region
        # (column-tile) needs its own start=True per tile.
        tctx = {}

        def get_ctx(t):
            if t not in tctx:
                tctx[t] = {
                    "av": avps.tile([128, 512], f32, tag="av", name="av_ps"),
                    "den": denps.tile([128, 512], f32, tag="den", name="den_ps"),
                }
            return tctx[t]

        def emit_scores(ent):
            t, gi, ngrp, chunks, is_diag = ent
            sts = {}
            for h in range(2):
                hp = slice(64 * h, 64 * h + 64)
                st = stps.tile([128, 2, 512], f32, tag="st", name=f"st{h}")
                for i, (ki, off) in enumerate(chunks):
                    nc.tensor.matmul(
                        st[:, i, off:512],
                        lhsT=kt_all[hp, 128 * ki : 128 * ki + 128],
                        rhs=qt_all[hp, 512 * t + off : 512 * t + 512],
                        start=True, stop=True,
                        tile_position=(64 * h, 0),
                    )
                sts[h] = st
            return sts

        def emit_p(ent, sts):
            t, gi, ngrp, chunks, is_diag = ent
            ps = {}
            for h in range(2):
                st = sts[h]
                p_sb = ppool.tile(
                    [128, 2, 512], bf16, tag=f"p{h}", name=f"p{h}", bufs=4
                )
                if is_diag:
                    # exp over one rectangle per pair (the [off0, off1)
                    # strip of the second chunk is stale-psum garbage that
                    # nothing downstream reads); h0 exact on ACT, h1
                    # Schraudolph on DVE (engine balance)
                    off0 = chunks[0][1]
                    if h == 0:
                        nc.scalar.activation(
                            p_sb[:, :, off0:512], st[:, :, off0:512],
                            Exp, scale=LN2 / 128.0,
                        )
                    else:
                        nc.vector.tensor_scalar(
                            p_sb[:, :, off0:512].bitcast(i16),
                            st[:, :, off0:512], C2, None, Add,
                        )
                    # causal mask on the [off, off+128) block
                    for i, (ki, off) in enumerate(chunks):
                        nc.vector.tensor_mul(
                            p_sb[:, i, off : off + 128],
                            p_sb[:, i, off : off + 128],
                            mask_sb[:, :],
                        )
                elif h == 0:
                    nc.scalar.activation(
                        p_sb[:, :, :], st[:, :, :], Exp, scale=LN2 / 128.0
                    )
                else:
                    # DVE Schraudolph: bf16 bits = int16(st + C2)
                    nc.vector.tensor_scalar(
                        p_sb[:, :, :].bitcast(i16), st[:, :, :],
                        C2, None, Add,
                    )
                ps[h] = p_sb
            return ps

        def emit_av(ent, ps):
            t, gi, ngrp, chunks, is_diag = ent
            ctx = get_ctx(t)
            av_ps, den_ps = ctx["av"], ctx["den"]
            last_gi = gi == ngrp - 1
            # den slot first: 4 concurrent M=32 ones-broadcast matmuls
            # (tile 0 has no off-diagonal groups, so rows 64..127 would
            # stay unwritten; fold both chunk-parities onto cols 0/32).
            for i, (ki, off) in enumerate(chunks):
                for h in range(2):
                    cp = 32 * (2 * i + h) if t > 0 else 32 * h
                    nc.tensor.matmul(
                        den_ps[cp : cp + 32, off:512],
                        lhsT=ones32[:, :],
                        rhs=ps[h][:, i, off:512],
                        start=not ctx.get(f"den{cp}", False),
                        stop=(last_gi and i == len(chunks) - 1 and h == 1),
                        tile_position=(0, cp),
                        skip_group_check=True,
                    )
                    ctx[f"den{cp}"] = True
            # AV: two heads column-tiled, concurrent per chunk
            for i, (ki, off) in enumerate(chunks):
                for h in range(2):
                    nc.tensor.matmul(
                        av_ps[64 * h : 64 * h + 64, off:512],
                        lhsT=v_all[:, ki, 64 * h : 64 * h + 64],
                        rhs=ps[h][:, i, off:512],
                        start=not ctx.get(f"av{h}", False),
                        stop=(last_gi and i == len(chunks) - 1 and h == 1),
                        tile_position=(0, 64 * h),
                        skip_group_check=True,
                    )
                    ctx[f"av{h}"] = True

        def finalize(t):
            """Evacuate AV (ACT), fold den quadrants, DMA-pack the two den
            rows to [128,8], exact reciprocal, DMA-broadcast back, one
            [128,512] normalize mul, one output DMA."""
            ns = slice(512 * t, 512 * t + 512)
            ctx = tctx.pop(t)
            av_ps, den_ps = ctx["av"], ctx["den"]
            av_sb = opool.tile([128, 512], f32, tag="avsb", name="av_sb")
            nc.scalar.activation(av_sb[:, :], av_ps[:, :], Ident)
            denf = opool.tile([64, 512], f32, tag="denf", name="denf")
            if t == 0:
                nc.vector.tensor_copy(denf[:, :], den_ps[0:64, :])
            else:
                den_hi = opool.tile([64, 512], f32, tag="denhi", name="den_hi")
                nc.scalar.activation(den_hi[:, :], den_ps[64:128, :], Ident)
                nc.vector.tensor_add(denf[:, :], den_ps[0:64, :], den_hi[:, :])
            # rows 0..31 of denf are 32 copies of den_h0, rows 32..63 den_h1
            dd = drs.tile([2, 512], f32, tag="dd", name="dd")
            for h in range(2):
                nc.gpsimd.dma_start(
                    out=dd[h : h + 1, :], in_=denf[32 * h : 32 * h + 1, :]
                )
            packed = opool.tile([128, 8], f32, tag="packed", name="packed")
            nc.gpsimd.dma_start(
                out=packed[:, :], in_=dd.rearrange("h (p x) -> (h p) x", p=64)
            )
            r_sb = opool.tile([128, 8], f32, tag="r_sb", name="r_sb")
            nc.vector.reciprocal(r_sb[:, :], packed[:, :])
            dd2 = drs.tile([2, 512], f32, tag="dd2", name="dd2")
            nc.gpsimd.dma_start(
                out=dd2.rearrange("h (p x) -> (h p) x", p=64), in_=r_sb[:, :]
            )
            rb = opool.tile([128, 512], f32, tag="rb", name="rb")
            for h in range(2):
                nc.gpsimd.dma_start(
                    out=rb[64 * h : 64 * h + 64, :],
                    in_=dd2[h : h + 1, :].to_broadcast([64, 512]),
                )
            out_t = opool.tile([128, 512], f32, tag="out", name="out_t")
            nc.vector.tensor_mul(out_t[:, :], av_sb[:, :], rb[:, :])
            nc.sync.dma_start(out=o[:, ns], in_=out_t[:, :])

        # ================= pipelined main loop =================
        emit_proj(0)
        sts = {0: emit_scores(flat[0]), 1: emit_scores(flat[1])}
        for j, ent in enumerate(flat):
            t, gi, ngrp = ent[0], ent[1], ent[2]
            # next tile's projections go in 3 groups before this tile ends,
            # ahead of the lookahead emission of the next tile's scores
            if gi == max(0, ngrp - 3) and t + 1 < NT:
                emit_proj(t + 1)
            cur_p = emit_p(ent, sts.pop(j))
            if j + 2 < len(flat):
                sts[j + 2] = emit_scores(flat[j + 2])
            emit_av(ent, cur_p)
            if gi == ngrp - 1:
                finalize(t)

    _split_multi_waits(nc, mybir, bass_rust)
    return nc


def kernel(query, key, Wq, bq, Wk, bk, Wv, bv):
    from concourse.bass_utils import run_bass_kernel_spmd

    global last_results
    if "nc" not in _cache:
        _cache["nc"] = _build_program()
    nc = _cache["nc"]

    query = np.asarray(query, np.float32)
    key = np.asarray(key, np.float32)
    Wq = np.asarray(Wq, np.float32)
    Wk = np.asarray(Wk, np.float32)
    Wv = np.asarray(Wv, np.float32)
    bq = np.asarray(bq, np.float32)
    bk = np.asarray(bk, np.float32)
    bv = np.asarray(bv, np.float32)

    # shared per-batch inputs
    xq_b = [query[b].reshape(C, N).astype(ml_dtypes.bfloat16) for b in range(B)]
    xk_b = [key[b].reshape(C, N).astype(ml_dtypes.bfloat16) for b in range(B)]

    # causal mask for the diagonal 128-block: mask[kk, qq] = qq >= kk
    kk = np.arange(128)[:, None]
    qq = np.arange(128)[None, :]
    mask = (qq >= kk).astype(ml_dtypes.bfloat16)

    qscale = C1 / 8.0  # 1/sqrt(hd) plus the exp prescale

    in_maps = []
    for core in range(8):
        b, p = core // 4, core % 4
        sl = slice(128 * p, 128 * p + 128)
        # [C, e] -> [c, p, e] -> [p, c, e] (contiguous per partition)
        wq_h = (Wq[sl] * qscale).T.reshape(CCH, 128, 128).transpose(1, 0, 2)
        wk_h = Wk[sl].T.reshape(CCH, 128, 128).transpose(1, 0, 2)
        wv_h = Wv[sl].T.reshape(CCH, 128, 128).transpose(1, 0, 2)
        wqkv = np.ascontiguousarray(
            np.stack([wq_h, wk_h, wv_h], axis=1)
        ).astype(ml_dtypes.bfloat16)
        smalls = np.ascontiguousarray(
            np.concatenate(
                [
                    (bq[sl] * qscale)[:, None],
                    bk[sl][:, None],
                    np.broadcast_to(bv[sl], (128, 128)),
                ],
                axis=1,
            )
        ).astype(np.float32)
        in_maps.append(
            {
                "xq": xq_b[b],
                "xk": xk_b[b],
                "wqkv": wqkv,
                "smalls": smalls,
                "mask": mask,
            }
        )

    trace = bool(int(os.environ.get("KERNEL_TRACE", "0")))
    res = run_bass_kernel_spmd(nc, in_maps, core_ids=list(range(8)), trace=trace)
    last_results = res

    out = np.empty((B, E, H, W), np.float32)
    for core in range(8):
        b, p = core // 4, core % 4
        out[b, 128 * p : 128 * p + 128] = res.results[core]["o"].reshape(128, H, W)
    return out


# revision 17
# speedup vs baseline: 1.3263x; 1.1218x over previous
"""CausalAttention2d Trainium2 kernel (8-core SPMD), v4.

Shards (B=2, heads=8) -> 16 (batch, head) pairs across 8 cores: core c handles
batch b=c//4 and head-pair p=c%4 (heads 2p, 2p+1 = E-channels 128p..128p+128).
Each core computes Q/K/V projections for its head pair, causal softmax
attention over the full N=4096 sequence, and writes its [128, 4096] slice of
the channel-major output. Host assembles the full (2, 512, 64, 64) tensor.

v4 changes over v3 (188us):
 - AV matmuls column-tiled: the two heads' AV run CONCURRENTLY on PE column
   halves (tile_position (0,0)/(0,64), M=64 each) into one [128,512] psum
   bank -> 216ns per key-chunk for BOTH heads (was 432, HW-verified).
 - Softmax denominators come from a dedicated den-slot: per group, four
   concurrent M=32 ones-broadcast matmuls (col positions 0/32/64/96) that
   accumulate 32 identical copies of each (head, chunk-parity) partial den
   into a second psum bank; one ACT cross-base evac + one DVE add folds the
   quadrants. Den slot runs BEFORE the AV slots so the last tile's
   denominator roundtrip overlaps its final AV matmuls.
 - Finalize: ACT (not DVE) evacuates AV psum; ONE [128,512] DVE mul
   normalizes both heads; one DMA per tile writes the output.
 - Input staging: biases/mask merged into single contiguous staging tensors
   and weights sent pre-permuted, killing the per-partition 4-byte DMA
   descriptor storm that delayed the first matmul to 14.5us.
 - ~30 tiny warm-up matmuls during the input-DMA wait pre-warm the PE HAM
   clock gate (cold K=4/8 -> warm K=8/8).
 - exp split tuned: head0 exact on ACT; head1 Schraudolph on DVE for cols
   [0:SPL2) of each 512-chunk, exact ACT tail for the rest (engine balance).
"""

import os

import numpy as np
import ml_dtypes

B, C, H, W = 2, 512, 64, 64
N = H * W          # 4096
E = 512
NH = 8
HD = 64
NT = 8             # n-tiles of 512
CCH = 4            # contraction chunks of 128 over C

LN2 = 0.6931471805599453
C1 = 128.0 / LN2           # score prescale (folded into Wq host-side)
C2 = 16248.75              # Schraudolph bias (calibrated, robust to rounding mode)
SPL2 = 480                 # head1 exp: DVE Schraudolph cols [0:SPL2), ACT tail

_cache = {}
last_results = None  # BassKernelResults of the most recent run (for profiling)


def _split_multi_waits(nc, mybir, bass_rust):
    """This walrus build accepts only ONE sync-wait per instruction; hoist
    extra waits onto single-wait NOPs inserted just before, preserving
    per-engine program order."""
    n = 0
    for f in nc.m.functions:
        for bb in f.blocks:
            old = list(bb.instructions)
            new = []
            changed = False
            for inst in old:
                si = inst.sync_info
                if si is not None and si.on_wait and len(si.on_wait) > 1:
                    waits = list(si.on_wait)
                    for w in waits[:-1]:
                        nop = bass_rust.InstNoOp(
                            name=f"I-sw{n}", engine=inst.engine, ins=[], outs=[]
                        )
                        n += 1
                        nop.sync_info = mybir.SyncInfo(on_wait=[w], on_update=[])
                        new.append(nop)
                    inst.sync_info = mybir.SyncInfo(
                        on_wait=[waits[-1]], on_update=list(si.on_update)
                    )
                    changed = True
                new.append(inst)
            if changed:
                bb.instructions = new
    return n


def _build_program():
    import bass_rust
    import concourse.bass as bass
    import concourse.mybir as mybir
    import concourse.tile as tile
    from contextlib import ExitStack

    f32 = mybir.dt.float32
    bf16 = mybir.dt.bfloat16
    i16 = mybir.dt.int16
    Exp = mybir.ActivationFunctionType.Exp
    Ident = mybir.ActivationFunctionType.Identity
    Add = mybir.AluOpType.add

    nc = bass.Bass()
    xq = nc.dram_tensor("xq", [C, N], bf16, kind="ExternalInput")
    xk = nc.dram_tensor("xk", [C, N], bf16, kind="ExternalInput")
    wqkv = nc.dram_tensor("wqkv", [128, 3, CCH, 128], bf16, kind="ExternalInput")
    smalls = nc.dram_tensor("smalls", [128, 130], f32, kind="ExternalInput")
    mask = nc.dram_tensor("mask", [128, 128], bf16, kind="ExternalInput")
    o = nc.dram_tensor("o", [128, N], f32, kind="ExternalOutput")

    with tile.TileContext(nc) as tc, ExitStack() as ctx:
        singles = ctx.enter_context(tc.tile_pool(name="singles", bufs=1))
        ppool = ctx.enter_context(tc.tile_pool(name="ppool", bufs=2))
        opool = ctx.enter_context(tc.tile_pool(name="opool", bufs=3))
        drs = ctx.enter_context(tc.tile_pool(name="drs", bufs=2, space="DRAM"))
        # PSUM budget (8 banks): st ring 3 bufs x 2 banks (scores AND
        # projection psums share it) + av 1 bank + den 1 bank.
        stps = ctx.enter_context(tc.tile_pool(name="stps", bufs=3, space="PSUM"))
        avps = ctx.enter_context(tc.tile_pool(name="avps", bufs=1, space="PSUM"))
        denps = ctx.enter_context(tc.tile_pool(name="denps", bufs=1, space="PSUM"))

        smalls_sb = singles.tile([128, 130], f32)
        mask_sb = singles.tile([128, 128], bf16)
        ones32 = singles.tile([128, 32], bf16)
        warm_sb = singles.tile([128, 16], bf16)
        # warm-up: force the exp ACT table set resident before the first
        # real activation (overlaps the input DMAs instead of serializing
        # into the first tile's dependency chain)
        warm = singles.tile([1, 8], f32)
        nc.vector.memset(warm, 0.0)
        nc.vector.memset(ones32, 1.0)
        nc.vector.memset(warm_sb, 1.0)
        nc.scalar.activation(warm[:, :], warm[:, :], Exp)
        nc.sync.dma_start(out=smalls_sb, in_=smalls[:, :])
        nc.sync.dma_start(out=mask_sb, in_=mask[:, :])
        bq_sb = smalls_sb[:, 0:1]
        bk_sb = smalls_sb[:, 1:2]
        bvb_sb = smalls_sb[:, 2:130]

        # persistent activations
        qt_all = singles.tile([128, N], bf16)   # [2*64 d, q] (prescaled by C1)
        kt_all = singles.tile([128, N], bf16)   # [2*64 d, keys]
        v_all = singles.tile([128, N // 128, 128], bf16)  # [keys, chunk, 2*64]

        # ---- weights (pre-permuted host-side: [p, which, c, e], contiguous)
        w_sb = singles.tile([128, 3, CCH, 128], bf16)
        nc.sync.dma_start(out=w_sb, in_=wqkv[:, :, :, :])
        xq_r = xq.rearrange("(c p) n -> p c n", p=128)
        xk_r = xk.rearrange("(c p) n -> p c n", p=128)
        xq_c = [singles.tile([128, N], bf16, name=f"xq_c{c}") for c in range(CCH)]
        xk_c = [singles.tile([128, N], bf16, name=f"xk_c{c}") for c in range(CCH)]

        for nq in (slice(0, 512), slice(512, 1024), slice(1024, 2048),
                   slice(2048, 4096)):
            for c in range(CCH):
                nc.sync.dma_start(out=xk_c[c][:, nq], in_=xk_r[:, c, nq])
                nc.gpsimd.dma_start(out=xq_c[c][:, nq], in_=xq_r[:, c, nq])

        # PE HAM pre-warm during the input-DMA wait: ~30 tiny matmuls keep
        # the PE busy so the clock gate opens to 8/8 before projections.
        warm_ps = stps.tile([16, 8], f32, tag="st", name="warm_ps")
        for i in range(30):
            nc.tensor.matmul(
                warm_ps[:, :], lhsT=warm_sb[:, :], rhs=warm_sb[:, 0:8],
                start=True, stop=True, skip_group_check=True,
            )

        def emit_proj(t):
            """Projections for tile t: qt/kt slices + v chunks."""
            ns = slice(512 * t, 512 * t + 512)
            qt_ps = stps.tile([128, 512], f32, tag="st", name="qt_ps")
            for c in range(CCH):
                nc.tensor.matmul(
                    qt_ps[:, :], lhsT=w_sb[:, 0, c, :], rhs=xq_c[c][:, ns],
                    start=(c == 0), stop=(c == CCH - 1),
                )
            nc.scalar.activation(qt_all[:, ns], qt_ps[:, :], Ident, bias=bq_sb)
            kt_ps = stps.tile([128, 512], f32, tag="st", name="kt_ps")
            for c in range(CCH):
                nc.tensor.matmul(
                    kt_ps[:, :], lhsT=w_sb[:, 1, c, :], rhs=xk_c[c][:, ns],
                    start=(c == 0), stop=(c == CCH - 1),
                )
            nc.scalar.activation(kt_all[:, ns], kt_ps[:, :], Ident, bias=bk_sb)
            for jj in range(4):
                i = 4 * t + jj
                nsj = slice(512 * t + 128 * jj, 512 * t + 128 * jj + 128)
                v_ps = stps.tile([128, 128], f32, tag="st", name="v_ps")
                for c in range(CCH):
                    nc.tensor.matmul(
                        v_ps[:, :],
                        lhsT=xk_c[c][:, nsj],
                        rhs=w_sb[:, 2, c, :],
                        start=(c == 0), stop=(c == CCH - 1),
                    )
                nc.vector.tensor_add(v_all[:, i, :], v_ps[:, :], bvb_sb)

        # ============== flat cross-tile group pipeline ==============
        # flat list of attention groups across ALL tiles; the lookahead-2
        # software pipeline runs over this list so the pipeline never breaks
        # at tile boundaries. Each entry: (t, gi, ngrp, chunks, is_diag)
        flat = []
        for t in range(NT):
            groups = []
            for g in range(2 * t):
                groups.append(([(2 * g, 0), (2 * g + 1, 0)], False))
            for d in range(2):
                groups.append(
                    ([(4 * t + r, 128 * r) for r in (2 * d, 2 * d + 1)], True)
                )
            for gi, (chunks, is_diag) in enumerate(groups):
                flat.append((t, gi, len(groups), chunks, is_diag))

        # per-tile psum contexts, created lazily
        # NOTE: start=True clears has_written only for the REGION the matmul
        # writes (not the whole bank), so every distinct output region
        # (column-tile) needs its own start=True per tile.
        tctx = {}

        def get_ctx(t):
            if t not in tctx:
                tctx[t] = {
                    "av": avps.tile([128, 512], f32, tag="av", name="av_ps"),
                    "den": denps.tile([128, 512], f32, tag="den", name="den_ps"),
                }
            return tctx[t]

        def emit_scores(ent):
            t, gi, ngrp, chunks, is_diag = ent
            sts = {}
            for h in range(2):
                hp = slice(64 * h, 64 * h + 64)
                st = stps.tile([128, 2, 512], f32, tag="st", name=f"st{h}")
                for i, (ki, off) in enumerate(chunks):
                    nc.tensor.matmul(
                        st[:, i, off:512],
                        lhsT=kt_all[hp, 128 * ki : 128 * ki + 128],
                        rhs=qt_all[hp, 512 * t + off : 512 * t + 512],
                        start=True, stop=True,
                        tile_position=(64 * h, 0),
                    )
                sts[h] = st
            return sts

        def emit_p(ent, sts):
            t, gi, ngrp, chunks, is_diag = ent
            ps = {}
            for h in range(2):
                st = sts[h]
                p_sb = ppool.tile(
                    [128, 2, 512], bf16, tag=f"p{h}", name=f"p{h}", bufs=4
                )
                if is_diag:
                    # exp over one rectangle per pair (the [off0, off1)
                    # strip of the second chunk is stale-psum garbage that
                    # nothing downstream reads); h0 exact on ACT, h1
                    # Schraudolph on DVE (engine balance)
                    off0 = chunks[0][1]
                    if h == 0:
                        nc.scalar.activation(
                            p_sb[:, :, off0:512], st[:, :, off0:512],
                            Exp, scale=LN2 / 128.0,
                        )
                    else:
                        nc.vector.tensor_scalar(
                            p_sb[:, :, off0:512].bitcast(i16),
                            st[:, :, off0:512], C2, None, Add,
                        )
                    # causal mask on the [off, off+128) block (GpSimd: all
                    # operands in SBUF, keeps ACT/DVE free for exp)
                    for i, (ki, off) in enumerate(chunks):
                        nc.gpsimd.tensor_mul(
                            p_sb[:, i, off : off + 128],
                            p_sb[:, i, off : off + 128],
                            mask_sb[:, :],
                        )
                elif h == 0:
                    nc.scalar.activation(
                        p_sb[:, :, :], st[:, :, :], Exp, scale=LN2 / 128.0
                    )
                else:
                    # DVE Schraudolph: bf16 bits = int16(st + C2)
                    nc.vector.tensor_scalar(
                        p_sb[:, :, :].bitcast(i16), st[:, :, :],
                        C2, None, Add,
                    )
                ps[h] = p_sb
            return ps

        def emit_av(ent, ps):
            t, gi, ngrp, chunks, is_diag = ent
            ctx = get_ctx(t)
            av_ps, den_ps = ctx["av"], ctx["den"]
            last_gi = gi == ngrp - 1
            # den slot first: 4 concurrent M=32 ones-broadcast matmuls
            # (tile 0 has no off-diagonal groups, so rows 64..127 would
            # stay unwritten; fold both chunk-parities onto cols 0/32).
            for i, (ki, off) in enumerate(chunks):
                for h in range(2):
                    cp = 32 * (2 * i + h) if t > 0 else 32 * h
                    nc.tensor.matmul(
                        den_ps[cp : cp + 32, off:512],
                        lhsT=ones32[:, :],
                        rhs=ps[h][:, i, off:512],
                        start=not ctx.get(f"den{cp}", False),
                        stop=(last_gi and i == len(chunks) - 1 and h == 1),
                        tile_position=(0, cp),
                        skip_group_check=True,
                    )
                    ctx[f"den{cp}"] = True
            # AV: two heads column-tiled, concurrent per chunk
            for i, (ki, off) in enumerate(chunks):
                for h in range(2):
                    nc.tensor.matmul(
                        av_ps[64 * h : 64 * h + 64, off:512],
                        lhsT=v_all[:, ki, 64 * h : 64 * h + 64],
                        rhs=ps[h][:, i, off:512],
                        start=not ctx.get(f"av{h}", False),
                        stop=(last_gi and i == len(chunks) - 1 and h == 1),
                        tile_position=(0, 64 * h),
                        skip_group_check=True,
                    )
                    ctx[f"av{h}"] = True

        def finalize(t):
            """Evacuate AV (ACT), fold den quadrants, DMA-pack the two den
            rows to [128,8], exact reciprocal, DMA-broadcast back, one
            [128,512] normalize mul, one output DMA."""
            ns = slice(512 * t, 512 * t + 512)
            ctx = tctx.pop(t)
            av_ps, den_ps = ctx["av"], ctx["den"]
            av_sb = opool.tile([128, 512], f32, tag="avsb", name="av_sb")
            nc.scalar.activation(av_sb[:, :], av_ps[:, :], Ident)
            denf = opool.tile([64, 512], f32, tag="denf", name="denf")
            if t == 0:
                nc.vector.tensor_copy(denf[:, :], den_ps[0:64, :])
            else:
                den_hi = opool.tile([64, 512], f32, tag="denhi", name="den_hi")
                nc.scalar.activation(den_hi[:, :], den_ps[64:128, :], Ident)
                nc.vector.tensor_add(denf[:, :], den_ps[0:64, :], den_hi[:, :])
            # rows 0..31 of denf are 32 copies of den_h0, rows 32..63 den_h1
            dd = drs.tile([2, 512], f32, tag="dd", name="dd")
            for h in range(2):
                nc.gpsimd.dma_start(
                    out=dd[h : h + 1, :], in_=denf[32 * h : 32 * h + 1, :]
                )
            packed = opool.tile([128, 8], f32, tag="packed", name="packed")
            nc.gpsimd.dma_start(
                out=packed[:, :], in_=dd.rearrange("h (p x) -> (h p) x", p=64)
            )
            r_sb = opool.tile([128, 8], f32, tag="r_sb", name="r_sb")
            nc.vector.reciprocal(r_sb[:, :], packed[:, :])
            dd2 = drs.tile([2, 512], f32, tag="dd2", name="dd2")
            nc.gpsimd.dma_start(
                out=dd2.rearrange("h (p x) -> (h p) x", p=64), in_=r_sb[:, :]
            )
            rb = opool.tile([128, 512], f32, tag="rb", name="rb")
            for h in range(2):
                nc.gpsimd.dma_start(
                    out=rb[64 * h : 64 * h + 64, :],
                    in_=dd2[h : h + 1, :].to_broadcast([64, 512]),
                )
            out_t = opool.tile([128, 512], f32, tag="out", name="out_t")
            nc.gpsimd.tensor_mul(out_t[:, :], av_sb[:, :], rb[:, :])
            nc.sync.dma_start(out=o[:, ns], in_=out_t[:, :])

        # ================= pipelined main loop =================
        # software pipeline over the flat group list with scores emitted 2
        # groups ahead (psum-ring permitting) and exp/p emitted 1 group
        # ahead, so the den/av matmuls' semaphore waits are pre-cleared.
        emit_proj(0)
        sts = {0: emit_scores(flat[0]), 1: emit_scores(flat[1])}
        pend = {0: emit_p(flat[0], sts.pop(0))}
        for j, ent in enumerate(flat):
            t, gi, ngrp = ent[0], ent[1], ent[2]
            # next tile's projections go in 3 groups before this tile ends,
            # ahead of the lookahead emission of the next tile's scores
            if gi == max(0, ngrp - 3) and t + 1 < NT:
                emit_proj(t + 1)
            cur_p = pend.pop(j)
            if j + 1 < len(flat):
                pend[j + 1] = emit_p(flat[j + 1], sts.pop(j + 1))
            if j + 2 < len(flat):
                sts[j + 2] = emit_scores(flat[j + 2])
            emit_av(ent, cur_p)
            if gi == ngrp - 1:
                finalize(t)

    _split_multi_waits(nc, mybir, bass_rust)
    return nc


def kernel(query, key, Wq, bq, Wk, bk, Wv, bv):
    from concourse.bass_utils import run_bass_kernel_spmd

    global last_results
    if "nc" not in _cache:
        _cache["nc"] = _build_program()
    nc = _cache["nc"]

    query = np.asarray(query, np.float32)
    key = np.asarray(key, np.float32)
    Wq = np.asarray(Wq, np.float32)
    Wk = np.asarray(Wk, np.float32)
    Wv = np.asarray(Wv, np.float32)
    bq = np.asarray(bq, np.float32)
    bk = np.asarray(bk, np.float32)
    bv = np.asarray(bv, np.float32)

    # shared per-batch inputs
    xq_b = [query[b].reshape(C, N).astype(ml_dtypes.bfloat16) for b in range(B)]
    xk_b = [key[b].reshape(C, N).astype(ml_dtypes.bfloat16) for b in range(B)]

    # causal mask for the diagonal 128-block: mask[kk, qq] = qq >= kk
    kk = np.arange(128)[:, None]
    qq = np.arange(128)[None, :]
    mask = (qq >= kk).astype(ml_dtypes.bfloat16)

    qscale = C1 / 8.0  # 1/sqrt(hd) plus the exp prescale

    in_maps = []
    for core in range(8):
        b, p = core // 4, core % 4
        sl = slice(128 * p, 128 * p + 128)
        # [C, e] -> [c, p, e] -> [p, c, e] (contiguous per partition)
        wq_h = (Wq[sl] * qscale).T.reshape(CCH, 128, 128).transpose(1, 0, 2)
        wk_h = Wk[sl].T.reshape(CCH, 128, 128).transpose(1, 0, 2)
        wv_h = Wv[sl].T.reshape(CCH, 128, 128).transpose(1, 0, 2)
        wqkv = np.ascontiguousarray(
            np.stack([wq_h, wk_h, wv_h], axis=1)
        ).astype(ml_dtypes.bfloat16)
        smalls = np.ascontiguousarray(
            np.concatenate(
                [
                    (bq[sl] * qscale)[:, None],
                    bk[sl][:, None],
                    np.broadcast_to(bv[sl], (128, 128)),
                ],
                axis=1,
            )
        ).astype(np.float32)
        in_maps.append(
            {
                "xq": xq_b[b],
                "xk": xk_b[b],
                "wqkv": wqkv,
                "smalls": smalls,
                "mask": mask,
            }
        )

    trace = bool(int(os.environ.get("KERNEL_TRACE", "0")))
    res = run_bass_kernel_spmd(nc, in_maps, core_ids=list(range(8)), trace=trace)
    last_results = res

    out = np.empty((B, E, H, W), np.float32)
    for core in range(8):
        b, p = core // 4, core % 4
        out[b, 128 * p : 128 * p + 128] = res.results[core]["o"].reshape(128, H, W)
    return out


# revision 27
# speedup vs baseline: 1.3435x; 1.0129x over previous
"""CausalAttention2d Trainium2 kernel (8-core SPMD), v4.

Shards (B=2, heads=8) -> 16 (batch, head) pairs across 8 cores: core c handles
batch b=c//4 and head-pair p=c%4 (heads 2p, 2p+1 = E-channels 128p..128p+128).
Each core computes Q/K/V projections for its head pair, causal softmax
attention over the full N=4096 sequence, and writes its [128, 4096] slice of
the channel-major output. Host assembles the full (2, 512, 64, 64) tensor.

v4 changes over v3 (188us):
 - AV matmuls column-tiled: the two heads' AV run CONCURRENTLY on PE column
   halves (tile_position (0,0)/(0,64), M=64 each) into one [128,512] psum
   bank -> 216ns per key-chunk for BOTH heads (was 432, HW-verified).
 - Softmax denominators come from a dedicated den-slot: per group, four
   concurrent M=32 ones-broadcast matmuls (col positions 0/32/64/96) that
   accumulate 32 identical copies of each (head, chunk-parity) partial den
   into a second psum bank; one ACT cross-base evac + one DVE add folds the
   quadrants. Den slot runs BEFORE the AV slots so the last tile's
   denominator roundtrip overlaps its final AV matmuls.
 - Finalize: ACT (not DVE) evacuates AV psum; ONE [128,512] DVE mul
   normalizes both heads; one DMA per tile writes the output.
 - Input staging: biases/mask merged into single contiguous staging tensors
   and weights sent pre-permuted, killing the per-partition 4-byte DMA
   descriptor storm that delayed the first matmul to 14.5us.
 - ~30 tiny warm-up matmuls during the input-DMA wait pre-warm the PE HAM
   clock gate (cold K=4/8 -> warm K=8/8).
 - exp split tuned: head0 exact on ACT; head1 Schraudolph on DVE for cols
   [0:SPL2) of each 512-chunk, exact ACT tail for the rest (engine balance).
"""

import os

import numpy as np
import ml_dtypes

B, C, H, W = 2, 512, 64, 64
N = H * W          # 4096
E = 512
NH = 8
HD = 64
NT = 8             # n-tiles of 512
CCH = 4            # contraction chunks of 128 over C

LN2 = 0.6931471805599453
C1 = 128.0 / LN2           # score prescale (folded into Wq host-side)
C2 = 16248.75              # Schraudolph bias (calibrated, robust to rounding mode)
SPL2 = 480                 # head1 exp: DVE Schraudolph cols [0:SPL2), ACT tail

_cache = {}
last_results = None  # BassKernelResults of the most recent run (for profiling)


def _split_multi_waits(nc, mybir, bass_rust):
    """This walrus build accepts only ONE sync-wait per instruction; hoist
    extra waits onto single-wait NOPs inserted just before, preserving
    per-engine program order."""
    n = 0
    for f in nc.m.functions:
        for bb in f.blocks:
            old = list(bb.instructions)
            new = []
            changed = False
            for inst in old:
                si = inst.sync_info
                if si is not None and si.on_wait and len(si.on_wait) > 1:
                    waits = list(si.on_wait)
                    for w in waits[:-1]:
                        nop = bass_rust.InstNoOp(
                            name=f"I-sw{n}", engine=inst.engine, ins=[], outs=[]
                        )
                        n += 1
                        nop.sync_info = mybir.SyncInfo(on_wait=[w], on_update=[])
                        new.append(nop)
                    inst.sync_info = mybir.SyncInfo(
                        on_wait=[waits[-1]], on_update=list(si.on_update)
                    )
                    changed = True
                new.append(inst)
            if changed:
                bb.instructions = new
    return n


def _build_program():
    import bass_rust
    import concourse.bass as bass
    import concourse.mybir as mybir
    import concourse.tile as tile
    from contextlib import ExitStack

    f32 = mybir.dt.float32
    bf16 = mybir.dt.bfloat16
    i16 = mybir.dt.int16
    i32 = mybir.dt.int32
    Exp = mybir.ActivationFunctionType.Exp
    Ident = mybir.ActivationFunctionType.Identity
    Add = mybir.AluOpType.add

    nc = bass.Bass()
    xq = nc.dram_tensor("xq", [C, N], bf16, kind="ExternalInput")
    xk = nc.dram_tensor("xk", [C, N], bf16, kind="ExternalInput")
    wqkv = nc.dram_tensor("wqkv", [128, 3, CCH, 128], bf16, kind="ExternalInput")
    smalls = nc.dram_tensor("smalls", [128, 130], f32, kind="ExternalInput")
    mask = nc.dram_tensor("mask", [128, 128], bf16, kind="ExternalInput")
    o = nc.dram_tensor("o", [128, N], f32, kind="ExternalOutput")

    with tile.TileContext(nc) as tc, ExitStack() as ctx:
        singles = ctx.enter_context(tc.tile_pool(name="singles", bufs=1))
        ppool = ctx.enter_context(tc.tile_pool(name="ppool", bufs=2))
        opool = ctx.enter_context(tc.tile_pool(name="opool", bufs=3))
        drs = ctx.enter_context(tc.tile_pool(name="drs", bufs=2, space="DRAM"))
        # PSUM budget (8 banks): st ring 3 bufs x 2 banks (scores AND
        # projection psums share it) + av 1 bank + den 1 bank.
        stps = ctx.enter_context(tc.tile_pool(name="stps", bufs=3, space="PSUM"))
        avps = ctx.enter_context(tc.tile_pool(name="avps", bufs=1, space="PSUM"))
        denps = ctx.enter_context(tc.tile_pool(name="denps", bufs=1, space="PSUM"))

        smalls_sb = singles.tile([128, 130], f32)
        mask_sb = singles.tile([128, 128], bf16)
        ones32 = singles.tile([128, 32], bf16)
        warm_sb = singles.tile([128, 16], bf16)
        magic_sb = singles.tile([64, 512], i32)
        nc.vector.memset(magic_sb, 0x7EF311C3)
        # warm-up: force the exp ACT table set resident before the first
        # real activation (overlaps the input DMAs instead of serializing
        # into the first tile's dependency chain)
        warm = singles.tile([1, 8], f32)
        nc.vector.memset(warm, 0.0)
        nc.vector.memset(ones32, 1.0)
        nc.vector.memset(warm_sb, 1.0)
        nc.scalar.activation(warm[:, :], warm[:, :], Exp)
        nc.sync.dma_start(out=smalls_sb, in_=smalls[:, :])
        nc.sync.dma_start(out=mask_sb, in_=mask[:, :])
        bq_sb = smalls_sb[:, 0:1]
        bk_sb = smalls_sb[:, 1:2]
        bvb_sb = smalls_sb[:, 2:130]

        # persistent activations
        qt_all = singles.tile([128, N], bf16)   # [2*64 d, q] (prescaled by C1)
        kt_all = singles.tile([128, N], bf16)   # [2*64 d, keys]
        v_all = singles.tile([128, N // 128, 128], bf16)  # [keys, chunk, 2*64]

        # ---- weights (pre-permuted host-side: [p, which, c, e], contiguous)
        w_sb = singles.tile([128, 3, CCH, 128], bf16)
        nc.sync.dma_start(out=w_sb, in_=wqkv[:, :, :, :])
        xq_r = xq.rearrange("(c p) n -> p c n", p=128)
        xk_r = xk.rearrange("(c p) n -> p c n", p=128)
        xq_c = [singles.tile([128, N], bf16, name=f"xq_c{c}") for c in range(CCH)]
        xk_c = [singles.tile([128, N], bf16, name=f"xk_c{c}") for c in range(CCH)]

        for nq in (slice(0, 512), slice(512, 1024), slice(1024, 2048),
                   slice(2048, 4096)):
            for c in range(CCH):
                nc.sync.dma_start(out=xk_c[c][:, nq], in_=xk_r[:, c, nq])
                nc.gpsimd.dma_start(out=xq_c[c][:, nq], in_=xq_r[:, c, nq])

        # PE HAM pre-warm during the input-DMA wait: ~30 tiny matmuls keep
        # the PE busy so the clock gate opens to 8/8 before projections.
        warm_ps = stps.tile([16, 8], f32, tag="st", name="warm_ps")
        for i in range(50):
            nc.tensor.matmul(
                warm_ps[:, :], lhsT=warm_sb[:, :], rhs=warm_sb[:, 0:8],
                start=True, stop=True, skip_group_check=True,
            )

        def emit_proj(t):
            """Projections for tile t: qt/kt slices + v chunks."""
            ns = slice(512 * t, 512 * t + 512)
            qt_ps = stps.tile([128, 512], f32, tag="st", name="qt_ps")
            for c in range(CCH):
                nc.tensor.matmul(
                    qt_ps[:, :], lhsT=w_sb[:, 0, c, :], rhs=xq_c[c][:, ns],
                    start=(c == 0), stop=(c == CCH - 1),
                )
            nc.scalar.activation(qt_all[:, ns], qt_ps[:, :], Ident, bias=bq_sb)
            kt_ps = stps.tile([128, 512], f32, tag="st", name="kt_ps")
            for c in range(CCH):
                nc.tensor.matmul(
                    kt_ps[:, :], lhsT=w_sb[:, 1, c, :], rhs=xk_c[c][:, ns],
                    start=(c == 0), stop=(c == CCH - 1),
                )
            nc.scalar.activation(kt_all[:, ns], kt_ps[:, :], Ident, bias=bk_sb)
            for jj in range(4):
                i = 4 * t + jj
                nsj = slice(512 * t + 128 * jj, 512 * t + 128 * jj + 128)
                v_ps = stps.tile([128, 128], f32, tag="st", name="v_ps")
                for c in range(CCH):
                    nc.tensor.matmul(
                        v_ps[:, :],
                        lhsT=xk_c[c][:, nsj],
                        rhs=w_sb[:, 2, c, :],
                        start=(c == 0), stop=(c == CCH - 1),
                    )
                nc.vector.tensor_add(v_all[:, i, :], v_ps[:, :], bvb_sb)

        # ============== flat cross-tile group pipeline ==============
        # flat list of attention groups across ALL tiles; the lookahead-2
        # software pipeline runs over this list so the pipeline never breaks
        # at tile boundaries. Each entry: (t, gi, ngrp, chunks, is_diag)
        flat = []
        for t in range(NT):
            groups = []
            for g in range(2 * t):
                groups.append(([(2 * g, 0), (2 * g + 1, 0)], False))
            for d in range(2):
                groups.append(
                    ([(4 * t + r, 128 * r) for r in (2 * d, 2 * d + 1)], True)
                )
            for gi, (chunks, is_diag) in enumerate(groups):
                flat.append((t, gi, len(groups), chunks, is_diag))

        # per-tile psum contexts, created lazily
        # NOTE: start=True clears has_written only for the REGION the matmul
        # writes (not the whole bank), so every distinct output region
        # (column-tile) needs its own start=True per tile.
        tctx = {}

        def get_ctx(t):
            if t not in tctx:
                tctx[t] = {
                    "av": avps.tile([128, 512], f32, tag="av", name="av_ps"),
                    "den": denps.tile([128, 512], f32, tag="den", name="den_ps"),
                }
            return tctx[t]

        def emit_scores(ent):
            t, gi, ngrp, chunks, is_diag = ent
            sts = {}
            for h in range(2):
                hp = slice(64 * h, 64 * h + 64)
                st = stps.tile([128, 2, 512], f32, tag="st", name=f"st{h}")
                for i, (ki, off) in enumerate(chunks):
                    nc.tensor.matmul(
                        st[:, i, off:512],
                        lhsT=kt_all[hp, 128 * ki : 128 * ki + 128],
                        rhs=qt_all[hp, 512 * t + off : 512 * t + 512],
                        start=True, stop=True,
                        tile_position=(64 * h, 0),
                    )
                sts[h] = st
            return sts

        def emit_p(ent, sts):
            t, gi, ngrp, chunks, is_diag = ent
            ps = {}
            for h in range(2):
                st = sts[h]
                p_sb = ppool.tile(
                    [128, 2, 512], bf16, tag=f"p{h}", name=f"p{h}", bufs=4
                )
                if is_diag:
                    # exp over one rectangle per pair (the [off0, off1)
                    # strip of the second chunk is stale-psum garbage that
                    # nothing downstream reads); h0 exact on ACT, h1
                    # Schraudolph on DVE (engine balance)
                    off0 = chunks[0][1]
                    if h == 0:
                        nc.scalar.activation(
                            p_sb[:, :, off0:512], st[:, :, off0:512],
                            Exp, scale=LN2 / 128.0,
                        )
                    else:
                        nc.vector.tensor_scalar(
                            p_sb[:, :, off0:512].bitcast(i16),
                            st[:, :, off0:512], C2, None, Add,
                        )
                    # causal mask on the [off, off+128) block (GpSimd: all
                    # operands in SBUF, keeps ACT/DVE free for exp)
                    for i, (ki, off) in enumerate(chunks):
                        nc.gpsimd.tensor_mul(
                            p_sb[:, i, off : off + 128],
                            p_sb[:, i, off : off + 128],
                            mask_sb[:, :],
                        )
                elif h == 0:
                    nc.scalar.activation(
                        p_sb[:, :, :], st[:, :, :], Exp, scale=LN2 / 128.0
                    )
                else:
                    # DVE Schraudolph: bf16 bits = int16(st + C2)
                    nc.vector.tensor_scalar(
                        p_sb[:, :, :].bitcast(i16), st[:, :, :],
                        C2, None, Add,
                    )
                ps[h] = p_sb
            return ps

        def emit_den_slot(ent, ps):
            t, gi, ngrp, chunks, is_diag = ent
            ctx = get_ctx(t)
            den_ps = ctx["den"]
            last_gi = gi == ngrp - 1
            # den slot: 4 concurrent M=32 ones-broadcast matmuls (tile 0 has
            # no off-diagonal groups, so rows 64..127 would stay unwritten;
            # fold both chunk-parities onto cols 0/32).
            for i, (ki, off) in enumerate(chunks):
                for h in range(2):
                    cp = 32 * (2 * i + h) if t > 0 else 32 * h
                    nc.tensor.matmul(
                        den_ps[cp : cp + 32, off:512],
                        lhsT=ones32[:, :],
                        rhs=ps[h][:, i, off:512],
                        start=not ctx.get(f"den{cp}", False),
                        stop=(last_gi and i == len(chunks) - 1 and h == 1),
                        tile_position=(0, cp),
                        skip_group_check=True,
                    )
                    ctx[f"den{cp}"] = True

        def emit_av_slot(ent, ps):
            t, gi, ngrp, chunks, is_diag = ent
            ctx = get_ctx(t)
            av_ps = ctx["av"]
            last_gi = gi == ngrp - 1
            # AV: two heads column-tiled, concurrent per chunk
            for i, (ki, off) in enumerate(chunks):
                for h in range(2):
                    nc.tensor.matmul(
                        av_ps[64 * h : 64 * h + 64, off:512],
                        lhsT=v_all[:, ki, 64 * h : 64 * h + 64],
                        rhs=ps[h][:, i, off:512],
                        start=not ctx.get(f"av{h}", False),
                        stop=(last_gi and i == len(chunks) - 1 and h == 1),
                        tile_position=(0, 64 * h),
                        skip_group_check=True,
                    )
                    ctx[f"av{h}"] = True

        def finalize_den(t):
            """Fold den quadrants; then either DMA-pack to [128,8] + exact
            reciprocal + DMA-broadcast back (overlapped tiles), or for the
            final tile a DMA-free bit-trick + Newton-Raphson reciprocal on
            DVE (the sign of the NR iterate is absorbed by the av
            evacuation's scale=-1)."""
            ctx = tctx[t]
            den_ps = ctx["den"]
            denf = opool.tile([64, 512], f32, tag="denf", name="denf")
            if t == 0:
                nc.vector.tensor_copy(denf[:, :], den_ps[0:64, :])
            else:
                den_hi = opool.tile([64, 512], f32, tag="denhi", name="den_hi")
                nc.scalar.activation(den_hi[:, :], den_ps[64:128, :], Ident)
                nc.vector.tensor_add(denf[:, :], den_ps[0:64, :], den_hi[:, :])
            # rows 0..31 of denf are 32 copies of den_h0, rows 32..63 den_h1
            if t == NT - 1:
                # y0 = bitcast(MAGIC - bits(x)); u = (x*y0 - 2)*y0 = -y1
                y0 = opool.tile([64, 512], f32, tag="y0", name="y0")
                nc.vector.tensor_sub(
                    y0[:, :].bitcast(i32), magic_sb[:, :], denf[:, :].bitcast(i32)
                )
                xy = opool.tile([64, 512], f32, tag="xy", name="xy")
                nc.vector.tensor_mul(xy[:, :], denf[:, :], y0[:, :])
                # u = -1/den lands in the den psum bank (free after the
                # fold): PSUM operands are exempt from the SB equal-base-
                # partition rule, so the normalize muls can read it
                # cross-base.
                nc.vector.scalar_tensor_tensor(
                    den_ps[0:64, :], xy[:, :], 2.0, y0[:, :],
                    mybir.AluOpType.subtract, mybir.AluOpType.mult,
                )
                return
            dd = drs.tile([2, 512], f32, tag="dd", name="dd")
            nc.sync.dma_start(out=dd[0:1, :], in_=denf[0:1, :])
            nc.gpsimd.dma_start(out=dd[1:2, :], in_=denf[32:33, :])
            packed = opool.tile([128, 8], f32, tag="packed", name="packed")
            nc.gpsimd.dma_start(
                out=packed[:, :], in_=dd.rearrange("h (p x) -> (h p) x", p=64)
            )
            r_sb = opool.tile([128, 8], f32, tag="r_sb", name="r_sb")
            nc.vector.reciprocal(r_sb[:, :], packed[:, :])
            r_row = drs.tile([2, 512], f32, tag="r_row", name="r_row")
            nc.gpsimd.dma_start(
                out=r_row.rearrange("h (p x) -> (h p) x", p=64), in_=r_sb[:, :]
            )
            rb = opool.tile([128, 512], f32, tag="rb", name="rb")
            nc.sync.dma_start(
                out=rb[0:64, :], in_=r_row[0:1, :].to_broadcast([64, 512])
            )
            nc.gpsimd.dma_start(
                out=rb[64:128, :], in_=r_row[1:2, :].to_broadcast([64, 512])
            )
            ctx["rb"] = rb

        def finalize_out(t):
            """Evacuate AV (ACT), normalize, write the output tile."""
            ns = slice(512 * t, 512 * t + 512)
            ctx = tctx.pop(t)
            av_ps = ctx["av"]
            av_sb = opool.tile([128, 512], f32, tag="avsb", name="av_sb")
            out_t = opool.tile([128, 512], f32, tag="out", name="out_t")
            if t == NT - 1:
                nc.scalar.activation(av_sb[:, :], av_ps[:, :], Ident, scale=-1.0)
                u = ctx["den"]  # u = -1/den copies in the den psum bank
                # h0 rows need u[0:32] (den_h0 copies), h1 rows u[32:64]
                for a in range(4):
                    nc.vector.tensor_mul(
                        out_t[32 * a : 32 * a + 32, :],
                        av_sb[32 * a : 32 * a + 32, :],
                        u[32 * (a // 2) : 32 * (a // 2) + 32, :],
                    )
            else:
                nc.scalar.activation(av_sb[:, :], av_ps[:, :], Ident)
                nc.gpsimd.tensor_mul(out_t[:, :], av_sb[:, :], ctx["rb"][:, :])
            nc.sync.dma_start(out=o[:, ns], in_=out_t[:, :])

        # ================= pipelined main loop =================
        # software pipeline over the flat group list with scores emitted 2
        # groups ahead (psum-ring permitting) and exp/p emitted 1 group
        # ahead, so the den/av matmuls' semaphore waits are pre-cleared.
        emit_proj(0)
        sts = {0: emit_scores(flat[0]), 1: emit_scores(flat[1])}
        pend = {0: emit_p(flat[0], sts.pop(0))}
        for j, ent in enumerate(flat):
            t, gi, ngrp = ent[0], ent[1], ent[2]
            # next tile's projections go in 3 groups before this tile ends,
            # ahead of the lookahead emission of the next tile's scores
            if gi == max(0, ngrp - 3) and t + 1 < NT:
                emit_proj(t + 1)
            cur_p = pend.pop(j)
            if j + 1 < len(flat):
                pend[j + 1] = emit_p(flat[j + 1], sts.pop(j + 1))
            if j + 2 < len(flat):
                sts[j + 2] = emit_scores(flat[j + 2])
            emit_den_slot(ent, cur_p)
            if gi == ngrp - 1:
                finalize_den(t)
            emit_av_slot(ent, cur_p)
            if gi == ngrp - 1:
                finalize_out(t)

    _split_multi_waits(nc, mybir, bass_rust)
    return nc


def kernel(query, key, Wq, bq, Wk, bk, Wv, bv):
    from concourse.bass_utils import run_bass_kernel_spmd

    global last_results
    if "nc" not in _cache:
        _cache["nc"] = _build_program()
    nc = _cache["nc"]

    query = np.asarray(query, np.float32)
    key = np.asarray(key, np.float32)
    Wq = np.asarray(Wq, np.float32)
    Wk = np.asarray(Wk, np.float32)
    Wv = np.asarray(Wv, np.float32)
    bq = np.asarray(bq, np.float32)
    bk = np.asarray(bk, np.float32)
    bv = np.asarray(bv, np.float32)

    # shared per-batch inputs
    xq_b = [query[b].reshape(C, N).astype(ml_dtypes.bfloat16) for b in range(B)]
    xk_b = [key[b].reshape(C, N).astype(ml_dtypes.bfloat16) for b in range(B)]

    # causal mask for the diagonal 128-block: mask[kk, qq] = qq >= kk
    kk = np.arange(128)[:, None]
    qq = np.arange(128)[None, :]
    mask = (qq >= kk).astype(ml_dtypes.bfloat16)

    qscale = C1 / 8.0  # 1/sqrt(hd) plus the exp prescale

    in_maps = []
    for core in range(8):
        b, p = core // 4, core % 4
        sl = slice(128 * p, 128 * p + 128)
        # [C, e] -> [c, p, e] -> [p, c, e] (contiguous per partition)
        wq_h = (Wq[sl] * qscale).T.reshape(CCH, 128, 128).transpose(1, 0, 2)
        wk_h = Wk[sl].T.reshape(CCH, 128, 128).transpose(1, 0, 2)
        wv_h = Wv[sl].T.reshape(CCH, 128, 128).transpose(1, 0, 2)
        wqkv = np.ascontiguousarray(
            np.stack([wq_h, wk_h, wv_h], axis=1)
        ).astype(ml_dtypes.bfloat16)
        smalls = np.ascontiguousarray(
            np.concatenate(
                [
                    (bq[sl] * qscale)[:, None],
                    bk[sl][:, None],
                    np.broadcast_to(bv[sl], (128, 128)),
                ],
                axis=1,
            )
        ).astype(np.float32)
        in_maps.append(
            {
                "xq": xq_b[b],
                "xk": xk_b[b],
                "wqkv": wqkv,
                "smalls": smalls,
                "mask": mask,
            }
        )

    trace = bool(int(os.environ.get("KERNEL_TRACE", "0")))
    res = run_bass_kernel_spmd(nc, in_maps, core_ids=list(range(8)), trace=trace)
    last_results = res

    out = np.empty((B, E, H, W), np.float32)
    for core in range(8):
        b, p = core // 4, core % 4
        out[b, 128 * p : 128 * p + 128] = res.results[core]["o"].reshape(128, H, W)
    return out
